# revision 3
# baseline (speedup 1.0000x reference)
"""Trainium2 Bass kernel v3: colorization via Chebyshev-accelerated Jacobi.

v3 over v2:
  - n=20 steps (rho=0.975), T=10 -> a single halo exchange
  - per-group PSUM banks (292-col matmuls, 8 rotating banks): PE stays hot,
    evacuation is fine-grained (Act scaled-copy + DVE/Pool add per group)
  - Act PSUM preload of c_t*b -> only 8 matmul terms (taps), start=False
  - setup overhaul: contiguous partition-major input DMAs; host-precomputed
    1/count and sentinel mask (kills all valid-mask multiplies); 3x3 box
    stats via separable row-sum + one tridiagonal PE matmul; squares/exp on
    Act; weight finalize split DVE/Pool
  - finale in fp16, fp16 output DMA
"""
import sys

sys.path.insert(0, "/opt/trn_rl_repo")

from dataclasses import dataclass

import numpy as np

import concourse.bass as bass
import concourse.bacc as bacc
import concourse.mybir as mybir
from concourse import tile

F32 = mybir.dt.float32
F16 = mybir.dt.float16

OFFSETS = [(-1, -1), (-1, 0), (-1, 1), (0, -1), (0, 1), (1, -1), (1, 0), (1, 1)]
MAT_IDX = {0: 0, 1: 1, -1: 2}
SENT = 30000.0  # luminance sentinel marking out-of-image pixels

YIQ2RGB = [
    [1.0, 0.9468822170900693, 0.6235565819861433],
    [1.0, -0.27478764629897834, -0.6356910791873801],
    [1.0, -1.1085450346420322, 1.7090069284064666],
]


# per-step omegas for the 3-term iteration, optimized offline (Krylov/Gram
# fit of the degree-18 polynomial against W^100 e0 on the actual instance,
# then validated in a full fp16 pipeline simulation: rgb rel err 5.4e-3)
OMS_OPT18 = [
    1.404064, 1.797732, 2.502787, 1.422837, 1.709504, 1.319234, 1.672824,
    1.213824, 1.865266, 1.482599, 3.084687, 1.583229, 1.485692, 1.638472,
    1.052554, 2.153161, 1.981092,
]


@dataclass(frozen=True)
class Params:
    H: int = 1024
    W: int = 1024
    ncores: int = 8
    n_iters: int = 20   # iteration steps (n=18+OMS_OPT18 is faster but its
                        # aggressively-fit polynomial is too sensitive to the
                        # HW affinity perturbation: 1.25e-2 measured vs the
                        # robust chebyshev n=20's 6.3e-3)
    rho: float = 0.975  # fallback chebyshev rho (used when OMS is None)
    T: int = 10         # ghost depth (iterations between halo exchanges)
    cpg: int = 126      # owned columns per partition-group
    ns: int = 2         # column-group sets for the tap multiplies
    act_preload: bool = True
    pool_tap: int = 6   # tap computed on Pool (dy==0 so it is last on PE)

    @property
    def rpc(self):
        return self.H // self.ncores

    @property
    def R(self):
        return self.rpc + 2 * self.T + 2

    @property
    def NG(self):
        return -(-self.W // self.cpg)

    @property
    def R2(self):
        return 2 * self.R

    @property
    def W2(self):
        return self.NG * self.R2


PADE = 4


def cheb_schedule(p: Params):
    if p.n_iters == len(OMS_OPT18) + 1:
        oms = list(OMS_OPT18)
    else:
        oms, om = [], 1.0
        for _t in range(2, p.n_iters + 1):
            om = 1.0 / (1.0 - 0.25 * p.rho * p.rho * om)
            oms.append(om)
    s_prev, s_cur = 1.0, 1.0
    sched = [(1.0, 1.0)]
    for om in oms:
        s_new = s_prev / (1.0 - om)
        a = s_new * om / s_cur
        c = s_cur
        sched.append((a, c))
        s_prev, s_cur = s_cur, s_new
    return sched, s_cur


def _sets(p: Params):
    base = p.NG // p.ns
    rem = p.NG % p.ns
    out, g0 = [], 0
    for s in range(p.ns):
        g1 = g0 + base + (1 if s < rem else 0)
        out.append((g0, g1))
        g0 = g1
    return out


def _chunks(width32: int, cap: int = 512):
    out, o = [], 0
    while o < width32:
        out.append((o, min(cap, width32 - o)))
        o += cap
    return out


def build(p: Params):
    nc = bacc.Bacc("TRN2", target_bir_lowering=False, debug=False,
                   num_devices=p.ncores)
    NG, R, R2, W2 = p.NG, p.R, p.R2, p.W2
    RPC, T = p.rpc, p.T
    W2P = W2 + 2 * PADE

    gray_d = nc.dram_tensor("gray", [128, NG, R, 3], F32, kind="ExternalInput")
    appx_d = nc.dram_tensor("appx", [128, NG, R, 3], F32, kind="ExternalInput")
    rcount_d = nc.dram_tensor("rcount", [128, NG, R], F32, kind="ExternalInput")
    vsent_d = nc.dram_tensor("vsent", [128, NG, R], F16, kind="ExternalInput")
    mats_d = nc.dram_tensor("mats", [4, 128, 128], F16, kind="ExternalInput")
    uhot_d = nc.dram_tensor("uhot", [128, 16], F32, kind="ExternalInput")
    out_d = nc.dram_tensor("out", [128, NG, RPC, 3], F16, kind="ExternalOutput")

    sets = _sets(p)
    sched, s_final = cheb_schedule(p)
    inner = lambda a: a[:, :, 1 : R - 1]

    with tile.TileContext(nc) as tc:
        with (
            tc.tile_pool(name="persist", bufs=1) as pers,
            tc.tile_pool(name="dram", bufs=1, space="DRAM") as dram,
        ):
            y32 = pers.tile([128, NG, R], F32)
            x2 = pers.tile([128, 2, W2P], F16)
            b16 = pers.tile([128, W2P], F16)
            wde = [pers.tile([128, W2], F16, name=f"wde{k}", tag=f"wde{k}")
                   for k in range(8)]
            mats = pers.tile([128, 4, 128], F16)
            uhot = pers.tile([128, 16], F32)
            xg_sb = pers.tile([128, p.ncores, 2, 2, NG, T, 2], F16)

            xbnd = dram.tile([128, 2, 2, NG, T, 2], F16)      # [side, buf, ...]
            xgath = dram.tile([p.ncores, 128, 2, 2, NG, T, 2], F16)

            for i in range(4):
                nc.scalar.dma_start(mats[:, i, :], mats_d[i])
            nc.scalar.dma_start(uhot[:], uhot_d[:])

            # ---------------- setup ----------------
            with tc.tile_pool(name="mid", bufs=1) as mid:
                notc = mid.tile([128, NG, R], F32)
                rc16 = mid.tile([128, NG, R], F32)
                vs16 = mid.tile([128, NG, R], F16)
                nc.scalar.dma_start(rc16[:], rcount_d[:])
                nc.scalar.dma_start(vs16[:], vsent_d[:])

                with tc.tile_pool(name="ph1", bufs=1) as ph1:
                    g32 = ph1.tile([128, NG, R, 3], F32)
                    a32 = ph1.tile([128, NG, R, 3], F32)
                    # chunked loads: the Act-issued hwdge queue spreads chunks
                    # across DMA engines; sync/gpsimd pin everything to DMA_0
                    qrr = [nc.scalar, nc.sync, nc.scalar, nc.gpsimd]
                    for g in range(NG):
                        qrr[g % 4].dma_start(g32[:, g], gray_d[:, g])
                        qrr[(g + 2) % 4].dma_start(a32[:, g], appx_d[:, g])

                    ya = ph1.tile([128, NG, R], F32)
                    t0 = ph1.tile([128, NG, R], F32)
                    t1 = ph1.tile([128, NG, R], F32)
                    t2 = ph1.tile([128, NG, R], F32)
                    s_abs = ph1.tile([128, NG, R], F32)
                    cmask = ph1.tile([128, NG, R], F32)

                    # y = (0.3 R + 0.59 G + 0.11 B)/255 for gray & appendix
                    for (src, dst) in ((g32, y32), (a32, ya)):
                        nc.vector.tensor_scalar_mul(t0[:], src[:, :, :, 0], 0.3 / 255.0)
                        nc.vector.scalar_tensor_tensor(
                            t0[:], src[:, :, :, 1], 0.59 / 255.0, t0[:],
                            mybir.AluOpType.mult, mybir.AluOpType.add)
                        nc.vector.scalar_tensor_tensor(
                            dst[:], src[:, :, :, 2], 0.11 / 255.0, t0[:],
                            mybir.AluOpType.mult, mybir.AluOpType.add)

                    dr = ph1.tile([128, NG, R], F32)
                    db = ph1.tile([128, NG, R], F32)
                    nc.vector.scalar_tensor_tensor(
                        dr[:], a32[:, :, :, 0], 1.0 / 255.0, ya[:],
                        mybir.AluOpType.mult, mybir.AluOpType.subtract)
                    nc.vector.scalar_tensor_tensor(
                        db[:], a32[:, :, :, 2], 1.0 / 255.0, ya[:],
                        mybir.AluOpType.mult, mybir.AluOpType.subtract)
                    nc.vector.tensor_sub(t1[:], g32[:, :, :, 0], a32[:, :, :, 0])
                    nc.scalar.activation(s_abs[:], t1[:], mybir.ActivationFunctionType.Abs)
                    for ch in (1, 2):
                        nc.vector.tensor_sub(t1[:], g32[:, :, :, ch], a32[:, :, :, ch])
                        nc.scalar.activation(t2[:], t1[:], mybir.ActivationFunctionType.Abs)
                        nc.vector.tensor_add(s_abs[:], s_abs[:], t2[:])
                    nc.vector.tensor_scalar(cmask[:], s_abs[:], 2.55, None, mybir.AluOpType.is_gt)
                    nc.vector.tensor_scalar(notc[:], s_abs[:], 2.55, None, mybir.AluOpType.is_le)

                    iA = ph1.tile([128, NG, R], F32)
                    qA = ph1.tile([128, NG, R], F32)
                    nc.vector.tensor_scalar_mul(t1[:], db[:], -0.27)
                    nc.vector.scalar_tensor_tensor(
                        iA[:], dr[:], 0.74, t1[:], mybir.AluOpType.mult, mybir.AluOpType.add)
                    nc.vector.tensor_scalar_mul(t1[:], db[:], 0.41)
                    nc.vector.scalar_tensor_tensor(
                        qA[:], dr[:], 0.48, t1[:], mybir.AluOpType.mult, mybir.AluOpType.add)
                    nc.vector.tensor_mul(iA[:], iA[:], cmask[:])
                    nc.vector.tensor_mul(qA[:], qA[:], cmask[:])

                    nc.vector.memset(b16[:], 0.0)
                    bview = b16[:, PADE : PADE + W2].rearrange(
                        "p (g r c) -> p g r c", g=NG, r=R, c=2)
                    nc.vector.tensor_copy(bview[:, :, 1 : R - 1, 0], inner(iA))
                    nc.vector.tensor_copy(bview[:, :, 1 : R - 1, 1], inner(qA))
                    nc.vector.memset(x2[:], 0.0)
                    nc.vector.tensor_copy(x2[:, 0, :], b16[:])

                # ---------------- affinity weights ----------------
                with (
                    tc.tile_pool(name="ph2", bufs=1) as ph2,
                    tc.tile_pool(name="pp0", bufs=1, space="PSUM") as pp0,
                ):
                    # sentineled luminance + partition-shifted planes
                    ys = ph2.tile([128, NG, R], F32)
                    yp = ph2.tile([128, NG, R], F32)
                    ym = ph2.tile([128, NG, R], F32)
                    nc.vector.tensor_add(ys[:], y32[:], vs16[:])
                    nc.vector.memset(yp[:], SENT)
                    nc.vector.memset(ym[:], SENT)
                    nc.sync.dma_start(yp[0:127], ys[1:128])
                    nc.gpsimd.dma_start(ym[1:128], ys[0:127])
                    ypl = {1: yp, 0: ys, -1: ym}

                    # 3x3 box sums of y and y^2 (separable: rows on DVE,
                    # columns via tridiagonal matmul on PE)
                    r3 = ph2.tile([128, NG, R], F32)
                    r3q = ph2.tile([128, NG, R], F32)
                    y2 = ph2.tile([128, NG, R], F32)
                    nc.scalar.square(y2[:], y32[:])
                    nc.vector.scalar_tensor_tensor(
                        inner(r3), y32[:, :, 0 : R - 2], 1.0, y32[:, :, 2 : R],
                        mybir.AluOpType.mult, mybir.AluOpType.add)
                    nc.vector.tensor_add(inner(r3), inner(r3), inner(y32))
                    nc.vector.scalar_tensor_tensor(
                        inner(r3q), y2[:, :, 0 : R - 2], 1.0, y2[:, :, 2 : R],
                        mybir.AluOpType.mult, mybir.AluOpType.add)
                    nc.vector.tensor_add(inner(r3q), inner(r3q), inner(y2))

                    S1 = ph2.tile([128, NG, R], F32)
                    S2 = ph2.tile([128, NG, R], F32)
                    nc.vector.memset(S1[:], 0.0)
                    nc.vector.memset(S2[:], 0.0)
                    sbank = [pp0.tile([128, 512], F32, name=f"sb{i}",
                                      tag=f"sb{i}") for i in range(8)]
                    mbox32 = ph2.tile([128, 128], F32)
                    nc.vector.tensor_copy(mbox32[:], mats[:, 3, :])
                    RIN = R - 2
                    for j, (srct, dstt) in enumerate(((r3, S1), (r3q, S2))):
                        for g in range(NG):
                            ps = sbank[(j * NG + g) % 8]
                            nc.tensor.matmul(
                                ps[:, :RIN], mbox32[:],
                                srct[:, g, 1 : R - 1],
                                start=True, stop=True)
                            nc.scalar.copy(dstt[:, g, 1 : R - 1],
                                           ps[:, :RIN])

                    # var = S2*rc - (S1*rc)^2 ; negivs = -1/max(0.6 var, 2e-6)
                    m = ph2.tile([128, NG, R], F32)
                    m2 = ph2.tile([128, NG, R], F32)
                    nc.vector.tensor_mul(inner(m), inner(S1), inner(rc16))
                    nc.scalar.square(inner(m2), inner(m))
                    var = S2
                    nc.vector.tensor_mul(inner(var), inner(S2), inner(rc16))
                    nc.vector.tensor_sub(inner(var), inner(var), inner(m2))
                    negivs = S1
                    nc.vector.tensor_scalar(
                        inner(negivs), inner(var), 2e-6 / 0.6, None,
                        mybir.AluOpType.max)
                    nc.vector.reciprocal(inner(negivs), inner(negivs))
                    nc.vector.tensor_scalar_mul(inner(negivs), inner(negivs),
                                                -1.0 / 0.6)

                    # per-tap masked exp weights (sentinel kills invalid taps)
                    def shifted(plane, dx):
                        return plane[:, :, 1 + dx : R - 1 + dx]

                    mk = [ph2.tile([128, NG, R], F16, name=f"mk{k}", tag=f"mk{k}")
                          for k in range(8)]
                    for k, (dx, dy) in enumerate(OFFSETS):
                        d = ph2.tile([128, NG, R], F32, tag="d", bufs=3)
                        e = ph2.tile([128, NG, R], F32, tag="e", bufs=3)
                        nc.vector.tensor_sub(inner(d), shifted(ypl[dy], dx), inner(y32))
                        nc.scalar.square(inner(d), inner(d))
                        nc.vector.tensor_mul(inner(e), inner(d), inner(negivs))
                        nc.scalar.activation(
                            inner(mk[k]), inner(e), mybir.ActivationFunctionType.Exp)

                    # wsum (DVE/Pool split), wnorm = notc/max(wsum,eps)
                    wsum = m
                    wsb = m2
                    nc.vector.tensor_add(inner(wsum), inner(mk[0]), inner(mk[1]))
                    nc.gpsimd.tensor_add(inner(wsb), inner(mk[4]), inner(mk[5]))
                    nc.vector.tensor_add(inner(wsum), inner(wsum), inner(mk[2]))
                    nc.gpsimd.tensor_add(inner(wsb), inner(wsb), inner(mk[6]))
                    nc.vector.tensor_add(inner(wsum), inner(wsum), inner(mk[3]))
                    nc.gpsimd.tensor_add(inner(wsb), inner(wsb), inner(mk[7]))
                    nc.vector.tensor_add(inner(wsum), inner(wsum), inner(wsb))
                    wnorm = var
                    nc.vector.tensor_scalar(
                        inner(wnorm), inner(wsum), 1e-30, None, mybir.AluOpType.max)
                    nc.vector.reciprocal(inner(wnorm), inner(wnorm))
                    nc.vector.tensor_mul(inner(wnorm), inner(wnorm), inner(notc))
                    # zero the weights of out-of-image columns (the shifted-
                    # frame recompute below would otherwise give them exp(0)=1)
                    vbin = y2  # dead, reuse
                    nc.vector.tensor_scalar(vbin[:], vs16[:], 0.5, None,
                                            mybir.AluOpType.is_le)
                    nc.vector.tensor_mul(inner(wnorm), inner(wnorm), inner(vbin))

                    # partition-shifted planes of negivs and wnorm; with these
                    # the pre-shifted weights wde_k[p] = w_k[p - dy] can be
                    # recomputed directly in the shifted frame -- no strided
                    # partition-shift DMAs of the weight tensors needed.
                    negP = r3    # dead; reuse
                    negM = r3q   # dead; reuse
                    wnP = m      # (wsum) dead; reuse
                    wnM = m2     # (wsb) dead; reuse
                    nc.vector.memset(negP[:], -1.0)
                    nc.vector.memset(negM[:], -1.0)
                    nc.vector.memset(wnP[:], 0.0)
                    nc.vector.memset(wnM[:], 0.0)
                    nc.sync.dma_start(negP[0:127], negivs[1:128])
                    nc.gpsimd.dma_start(negM[1:128], negivs[0:127])
                    nc.sync.dma_start(wnP[0:127], wnorm[1:128])
                    nc.gpsimd.dma_start(wnM[1:128], wnorm[0:127])

                    # finalize: wde_k = dup(w~_k) with the partition pre-shift
                    # folded into the computation (center frame for dy==0,
                    # shifted frames for dy=+-1)
                    for k, (dx, dy) in enumerate(OFFSETS):
                        nc.vector.memset(wde[k][:], 0.0)
                        wv = wde[k][:].rearrange("p (g r c) -> p g r c",
                                                 g=NG, r=R, c=2)
                        if dy == 0:
                            mks, wn = mk[k], wnorm
                        else:
                            # frame shifted by -dy: center luma/params come
                            # from the opposite-shift planes
                            ctr = ym if dy == 1 else yp
                            ngv = negM if dy == 1 else negP
                            wn = wnM if dy == 1 else wnP
                            d = ph2.tile([128, NG, R], F32, tag="d", bufs=3)
                            e = ph2.tile([128, NG, R], F32, tag="e", bufs=3)
                            nc.vector.tensor_sub(inner(d), shifted(ys, dx),
                                                 inner(ctr))
                            nc.scalar.square(inner(d), inner(d))
                            nc.vector.tensor_mul(inner(e), inner(d), inner(ngv))
                            mks = ph2.tile([128, NG, R], F16, tag="mks", bufs=2)
                            nc.scalar.activation(
                                inner(mks), inner(e),
                                mybir.ActivationFunctionType.Exp)
                        nc.vector.tensor_mul(wv[:, :, 1 : R - 1, 0],
                                             inner(mks), inner(wn))
                        nc.gpsimd.tensor_mul(wv[:, :, 1 : R - 1, 1],
                                             inner(mks), inner(wn))

            # ---------------- Chebyshev iterations ----------------
            # PE term order: dy=-1, dy=+1 taps (DVE), then dy=0 with the Pool
            # tap last so PE never stalls on Pool early in a group.
            korder = [k for k, (dx, dy) in enumerate(OFFSETS) if dy == -1]
            korder += [k for k, (dx, dy) in enumerate(OFFSETS) if dy == 1]
            korder += [k for k, (dx, dy) in enumerate(OFFSETS)
                       if dy == 0 and k != p.pool_tap]
            korder += [p.pool_tap]
            PRANGE = {0: (0, 127), -1: (0, 127), 1: (0, 128)}

            def xview(buf, a, b):
                return x2[:, buf, PADE + a : PADE + b].rearrange(
                    "p (g r c) -> p g r c", g=(b - a) // R2, r=R, c=2)

            with (
                tc.tile_pool(name="qp", bufs=1) as qp,
                tc.tile_pool(name="pp", bufs=1, space="PSUM") as pp,
            ):
                banks = [pp.tile([128, 512], F32, name=f"bank{i}", tag=f"bank{i}")
                         for i in range(8)]
                qtiles = []
                for si, (g0, g1) in enumerate(sets):
                    sw = (g1 - g0) * R2
                    row = []
                    for k in range(8):
                        qt = qp.tile([128, sw], F16, name=f"qt{si}_{k}",
                                     tag=f"qt{si}_{k}")
                        nc.vector.memset(qt[:], 0.0)
                        row.append(qt)
                    qtiles.append(row)

                set_of_g = {}
                for si, (g0, g1) in enumerate(sets):
                    for g in range(g0, g1):
                        set_of_g[g] = si

                for it in range(p.n_iters):
                    t = it + 1
                    a_t, c_t = sched[it]
                    src = it % 2
                    dst = 1 - src

                    # Act: preload c_t * b into banks 0..7 (groups 0..7);
                    # group 8 shares bank 0 and is preloaded after group 0's
                    # evacuation inside the per-group loop below.
                    def preload(g):
                        nc.scalar.mul(
                            banks[g % 8][:, :R2],
                            b16[:, PADE + g * R2 : PADE + (g + 1) * R2],
                            float(c_t))

                    for g in range(min(NG, 8)):
                        preload(g)

                    # tap multiplies (per set)
                    for si, (g0, g1) in enumerate(sets):
                        lo2, hi2 = g0 * R2, g1 * R2
                        for k in korder:
                            dx, dy = OFFSETS[k]
                            qt = qtiles[si][k]
                            pa, pb = PRANGE[dy]
                            eng = nc.gpsimd if k == p.pool_tap else nc.vector
                            eng.tensor_mul(
                                qt[pa:pb],
                                wde[k][pa:pb, lo2:hi2],
                                x2[pa:pb, src, PADE + lo2 + 2 * dx : PADE + hi2 + 2 * dx],
                            )

                    # per-group: 8 matmuls -> Act scaled evac -> add -> guards
                    for g in range(NG):
                        if g >= 8:
                            preload(g)
                        si = set_of_g[g]
                        g0 = sets[si][0]
                        ps = banks[g % 8]
                        qoff = (g - g0) * R2
                        for ti, k in enumerate(korder):
                            nc.tensor.matmul(
                                ps[:, :R2], mats[:, MAT_IDX[OFFSETS[k][1]], :],
                                qtiles[si][k][:, qoff : qoff + R2],
                                start=False, stop=(ti == len(korder) - 1))
                        pv = ps[:, :R2].rearrange("p (r c) -> p r c", r=R, c=2)
                        ta = qp.tile([128, R2], F16, tag=f"ta{g % 4}", bufs=2)
                        tav = ta[:].rearrange("p (r c) -> p r c", r=R, c=2)
                        nc.scalar.mul(tav[:, 1 : R - 1, :], pv[:, 1 : R - 1, :],
                                      float(a_t))
                        dvw = xview(dst, g * R2, (g + 1) * R2)[:, 0]
                        eng = nc.vector if g < 5 else nc.gpsimd
                        eng.tensor_add(dvw[:, 1 : R - 1, :], tav[:, 1 : R - 1, :],
                                       dvw[:, 1 : R - 1, :])
                        # guard refresh with the left neighbor group
                        if g >= 1:
                            lf = xview(dst, (g - 1) * R2, g * R2)[:, 0]
                            nc.sync.dma_start(dvw[0:1, 1 : R - 1, :],
                                              lf[126:127, 1 : R - 1, :])
                            nc.gpsimd.dma_start(lf[127:128, 1 : R - 1, :],
                                                dvw[1:2, 1 : R - 1, :])

                    # halo exchange every T steps (both buffers)
                    if t % T == 0 and t < p.n_iters:
                        xr = x2[:, :, PADE : PADE + W2].rearrange(
                            "p b (g r c) -> p b g r c", g=NG, r=R, c=2)
                        for b_ in range(2):
                            nc.sync.dma_start(
                                xbnd[:, 0, b_], xr[:, b_, :, T + 1 : 2 * T + 1, :])
                            nc.scalar.dma_start(
                                xbnd[:, 1, b_], xr[:, b_, :, RPC + 1 : RPC + T + 1, :])
                        nc.gpsimd.collective_compute(
                            "AllGather",
                            mybir.AluOpType.bypass,
                            replica_groups=[list(range(p.ncores))],
                            ins=[xbnd.opt()],
                            outs=[xgath.opt()],
                        )
                        for r in range(p.ncores):
                            (nc.sync if r % 2 == 0 else nc.scalar).dma_start(
                                xg_sb[:, r], xgath[r])
                        for side, ucol in ((1, 0), (0, 8)):
                            for b_ in range(2):
                                if side == 1:
                                    dst_v = xr[:, b_, :, 1 : T + 1, :]
                                else:
                                    dst_v = xr[:, b_, :, RPC + T + 1 : RPC + 2 * T + 1, :]
                                nc.vector.tensor_scalar_mul(
                                    dst_v, xg_sb[:, 0, side, b_],
                                    uhot[:, ucol : ucol + 1])
                                for r in range(1, p.ncores):
                                    nc.vector.scalar_tensor_tensor(
                                        dst_v, xg_sb[:, r, side, b_],
                                        uhot[:, ucol + r : ucol + r + 1], dst_v,
                                        mybir.AluOpType.mult, mybir.AluOpType.add)

            # -------------- output: yiq2rgb on owned rows (fp16) --------------
            with tc.tile_pool(name="ph3", bufs=1) as ph3:
                o16 = ph3.tile([128, NG, RPC, 3], F16)
                y255 = ph3.tile([128, NG, RPC], F16)
                t3a = ph3.tile([128, NG, RPC], F16)
                fbuf = p.n_iters % 2
                xv = x2[:, fbuf, PADE : PADE + W2].rearrange(
                    "p (g r c) -> p g r c", g=NG, r=R, c=2)
                xi = xv[:, :, T + 1 : T + 1 + RPC, 0]
                xq = xv[:, :, T + 1 : T + 1 + RPC, 1]
                yo = y32[:, :, T + 1 : T + 1 + RPC]
                nc.scalar.mul(y255[:], yo, 255.0)
                inv_s = 255.0 / s_final
                for ch in range(3):
                    cy, ci, cq = YIQ2RGB[ch]
                    nc.vector.scalar_tensor_tensor(
                        t3a[:], xi, ci * inv_s, y255[:],
                        mybir.AluOpType.mult, mybir.AluOpType.add)
                    nc.vector.scalar_tensor_tensor(
                        t3a[:], xq, cq * inv_s, t3a[:],
                        mybir.AluOpType.mult, mybir.AluOpType.add)
                    nc.vector.tensor_scalar(
                        o16[:, :, :, ch], t3a[:], 0.0, 255.0,
                        mybir.AluOpType.max, mybir.AluOpType.min)
                nc.sync.dma_start(out_d[:], o16[:])

    nc.compile()
    return nc


# ---------------------------------------------------------------------------
# host-side sharding / assembly
# ---------------------------------------------------------------------------

def host_inputs(p: Params, gray: np.ndarray, appx: np.ndarray):
    H, W, T, NG, R, RPC = p.H, p.W, p.T, p.NG, p.R, p.rpc
    colw = p.cpg * NG + 2
    rpad = T + 1

    def padimg(img):
        return np.pad(
            img.astype(np.float32),
            ((rpad, R), (1, colw - 1 - W), (0, 0)),
        )

    gpad = padimg(gray)
    apad = padimg(appx)
    vpad = np.pad(np.ones((H, W), np.float32), ((rpad, R), (1, colw - 1 - W)))

    # count over the 3x3 box (valid neighbors + center)
    from numpy.lib.stride_tricks import sliding_window_view
    vp2 = np.pad(vpad, 1)
    cnt = sliding_window_view(vp2, (3, 3)).sum(axis=(2, 3))
    rcount_full = (1.0 / np.maximum(cnt, 1.0)).astype(np.float32)
    vsent_full = ((1.0 - vpad) * SENT).astype(np.float16)

    M = np.zeros((4, 128, 128), np.float16)
    for pp_ in range(1, 127):
        M[0, pp_, pp_] = 1
        M[1, pp_ + 1, pp_] = 1
        M[2, pp_ - 1, pp_] = 1
        M[3, pp_ - 1, pp_] = 1
        M[3, pp_, pp_] = 1
        M[3, pp_ + 1, pp_] = 1

    in_maps = []
    for c in range(p.ncores):
        r0 = RPC * c
        gT = np.empty((128, NG, R, 3), np.float32)
        aT = np.empty((128, NG, R, 3), np.float32)
        rT = np.empty((128, NG, R), np.float32)
        vT = np.empty((128, NG, R), np.float16)
        for g in range(NG):
            c0 = p.cpg * g
            gT[:, g] = gpad[r0 : r0 + R, c0 : c0 + 128].transpose(1, 0, 2)
            aT[:, g] = apad[r0 : r0 + R, c0 : c0 + 128].transpose(1, 0, 2)
            rT[:, g] = rcount_full[r0 : r0 + R, c0 : c0 + 128].T
            vT[:, g] = vsent_full[r0 : r0 + R, c0 : c0 + 128].T
        uhot = np.zeros((128, 16), np.float32)
        uhot[:, (c - 1) % p.ncores] = 1
        uhot[:, 8 + (c + 1) % p.ncores] = 1
        in_maps.append({"gray": np.ascontiguousarray(gT),
                        "appx": np.ascontiguousarray(aT),
                        "rcount": np.ascontiguousarray(rT),
                        "vsent": np.ascontiguousarray(vT),
                        "mats": M, "uhot": uhot})
    return in_maps


def assemble(p: Params, results):
    img = np.zeros((p.H, p.W, 3), np.float32)
    for c in range(p.ncores):
        o = np.asarray(results[c]["out"]).astype(np.float32)
        r0 = p.rpc * c
        for g in range(p.NG):
            ncols = min(p.cpg, p.W - p.cpg * g)
            img[r0 : r0 + p.rpc, p.cpg * g : p.cpg * g + ncols] = (
                o[1 : 1 + ncols, g].transpose(1, 0, 2))
    return img


# ---------------------------------------------------------------------------
# entry point
# ---------------------------------------------------------------------------

_CACHE = {}


def _get_program(p: Params):
    if p not in _CACHE:
        _CACHE[p] = build(p)
    return _CACHE[p]


def kernel(gray_rgb: np.ndarray, appendix_rgb: np.ndarray) -> np.ndarray:
    from concourse.bass_utils import run_bass_kernel_spmd

    p = Params()
    nc = _get_program(p)
    in_maps = host_inputs(p, np.asarray(gray_rgb), np.asarray(appendix_rgb))
    res = run_bass_kernel_spmd(nc, in_maps, list(range(p.ncores)))
    return assemble(p, res.results)


# revision 4
# speedup vs baseline: 1.0528x; 1.0528x over previous
"""Trainium2 Bass kernel v3: colorization via Chebyshev-accelerated Jacobi.

v3 over v2:
  - n=20 steps (rho=0.975), T=10 -> a single halo exchange
  - per-group PSUM banks (292-col matmuls, 8 rotating banks): PE stays hot,
    evacuation is fine-grained (Act scaled-copy + DVE/Pool add per group)
  - Act PSUM preload of c_t*b -> only 8 matmul terms (taps), start=False
  - setup overhaul: contiguous partition-major input DMAs; host-precomputed
    1/count and sentinel mask (kills all valid-mask multiplies); 3x3 box
    stats via separable row-sum + one tridiagonal PE matmul; squares/exp on
    Act; weight finalize split DVE/Pool
  - finale in fp16, fp16 output DMA
"""
import sys

sys.path.insert(0, "/opt/trn_rl_repo")

from dataclasses import dataclass

import numpy as np

import concourse.bass as bass
import concourse.bacc as bacc
import concourse.mybir as mybir
from concourse import tile

F32 = mybir.dt.float32
F16 = mybir.dt.float16

OFFSETS = [(-1, -1), (-1, 0), (-1, 1), (0, -1), (0, 1), (1, -1), (1, 0), (1, 1)]
MAT_IDX = {0: 0, 1: 1, -1: 2}
SENT = 30000.0  # luminance sentinel marking out-of-image pixels

YIQ2RGB = [
    [1.0, 0.9468822170900693, 0.6235565819861433],
    [1.0, -0.27478764629897834, -0.6356910791873801],
    [1.0, -1.1085450346420322, 1.7090069284064666],
]


# per-step omegas for the 3-term iteration, optimized offline (Krylov/Gram
# fit of the degree-18 polynomial against W^100 e0 on the actual instance,
# then validated in a full fp16 pipeline simulation: rgb rel err 5.4e-3)
OMS_OPT18 = [
    1.404064, 1.797732, 2.502787, 1.422837, 1.709504, 1.319234, 1.672824,
    1.213824, 1.865266, 1.482599, 3.084687, 1.583229, 1.485692, 1.638472,
    1.052554, 2.153161, 1.981092,
]


@dataclass(frozen=True)
class Params:
    H: int = 1024
    W: int = 1024
    ncores: int = 8
    n_iters: int = 20   # iteration steps (n=18+OMS_OPT18 is faster but its
                        # aggressively-fit polynomial is too sensitive to the
                        # HW affinity perturbation: 1.25e-2 measured vs the
                        # robust chebyshev n=20's 6.3e-3)
    rho: float = 0.975  # fallback chebyshev rho (used when OMS is None)
    T: int = 10         # ghost depth (iterations between halo exchanges)
    cpg: int = 126      # owned columns per partition-group
    ns: int = 2         # column-group sets for the tap multiplies
    act_preload: bool = True
    pool_tap: int = 6   # tap computed on Pool (dy==0 so it is last on PE)

    @property
    def rpc(self):
        return self.H // self.ncores

    @property
    def R(self):
        return self.rpc + 2 * self.T + 2

    @property
    def NG(self):
        return -(-self.W // self.cpg)

    @property
    def R2(self):
        return 2 * self.R

    @property
    def W2(self):
        return self.NG * self.R2


PADE = 4


def cheb_schedule(p: Params):
    if p.n_iters == len(OMS_OPT18) + 1:
        oms = list(OMS_OPT18)
    else:
        oms, om = [], 1.0
        for _t in range(2, p.n_iters + 1):
            om = 1.0 / (1.0 - 0.25 * p.rho * p.rho * om)
            oms.append(om)
    s_prev, s_cur = 1.0, 1.0
    sched = [(1.0, 1.0)]
    for om in oms:
        s_new = s_prev / (1.0 - om)
        a = s_new * om / s_cur
        c = s_cur
        sched.append((a, c))
        s_prev, s_cur = s_cur, s_new
    return sched, s_cur


def _sets(p: Params):
    base = p.NG // p.ns
    rem = p.NG % p.ns
    out, g0 = [], 0
    for s in range(p.ns):
        g1 = g0 + base + (1 if s < rem else 0)
        out.append((g0, g1))
        g0 = g1
    return out


def _chunks(width32: int, cap: int = 512):
    out, o = [], 0
    while o < width32:
        out.append((o, min(cap, width32 - o)))
        o += cap
    return out


def build(p: Params):
    nc = bacc.Bacc("TRN2", target_bir_lowering=False, debug=False,
                   num_devices=p.ncores)
    NG, R, R2, W2 = p.NG, p.R, p.R2, p.W2
    RPC, T = p.rpc, p.T
    W2P = W2 + 2 * PADE

    gray_d = nc.dram_tensor("gray", [128, NG, R, 3], F16, kind="ExternalInput")
    appx_d = nc.dram_tensor("appx", [128, NG, R, 3], F16, kind="ExternalInput")
    rcount_d = nc.dram_tensor("rcount", [128, NG, R], F32, kind="ExternalInput")
    vsent_d = nc.dram_tensor("vsent", [128, NG, R], F16, kind="ExternalInput")
    mats_d = nc.dram_tensor("mats", [4, 128, 128], F16, kind="ExternalInput")
    uhot_d = nc.dram_tensor("uhot", [128, 16], F32, kind="ExternalInput")
    out_d = nc.dram_tensor("out", [128, NG, RPC, 3], F16, kind="ExternalOutput")

    sets = _sets(p)
    sched, s_final = cheb_schedule(p)
    inner = lambda a: a[:, :, 1 : R - 1]

    with tile.TileContext(nc) as tc:
        with (
            tc.tile_pool(name="persist", bufs=1) as pers,
            tc.tile_pool(name="dram", bufs=1, space="DRAM") as dram,
        ):
            y32 = pers.tile([128, NG, R], F32)
            x2 = pers.tile([128, 2, W2P], F16)
            b16 = pers.tile([128, W2P], F16)
            wde = [pers.tile([128, W2], F16, name=f"wde{k}", tag=f"wde{k}")
                   for k in range(8)]
            mats = pers.tile([128, 4, 128], F16)
            uhot = pers.tile([128, 16], F32)
            xg_sb = pers.tile([128, p.ncores, 2, 2, NG, T, 2], F16)

            xbnd = dram.tile([128, 2, 2, NG, T, 2], F16)      # [side, buf, ...]
            xgath = dram.tile([p.ncores, 128, 2, 2, NG, T, 2], F16)

            for i in range(4):
                nc.scalar.dma_start(mats[:, i, :], mats_d[i])
            nc.scalar.dma_start(uhot[:], uhot_d[:])

            # ---------------- setup ----------------
            with tc.tile_pool(name="mid", bufs=1) as mid:
                notc = mid.tile([128, NG, R], F32)
                rc16 = mid.tile([128, NG, R], F32)
                vs16 = mid.tile([128, NG, R], F16)
                nc.scalar.dma_start(rc16[:], rcount_d[:])
                nc.scalar.dma_start(vs16[:], vsent_d[:])

                with tc.tile_pool(name="ph1", bufs=1) as ph1:
                    g32 = ph1.tile([128, NG, R, 3], F16)
                    a32 = ph1.tile([128, NG, R, 3], F16)
                    # chunked loads: the Act-issued hwdge queue spreads chunks
                    # across DMA engines; sync/gpsimd pin everything to DMA_0
                    qrr = [nc.scalar, nc.sync, nc.scalar, nc.gpsimd]
                    for g in range(NG):
                        qrr[g % 4].dma_start(g32[:, g], gray_d[:, g])
                        qrr[(g + 2) % 4].dma_start(a32[:, g], appx_d[:, g])

                    ya = ph1.tile([128, NG, R], F32)
                    t0 = ph1.tile([128, NG, R], F32)
                    t1 = ph1.tile([128, NG, R], F32)
                    t2 = ph1.tile([128, NG, R], F32)
                    s_abs = ph1.tile([128, NG, R], F32)
                    cmask = ph1.tile([128, NG, R], F32)

                    # y = (0.3 R + 0.59 G + 0.11 B)/255 for gray & appendix
                    for (src, dst) in ((g32, y32), (a32, ya)):
                        nc.vector.tensor_scalar_mul(t0[:], src[:, :, :, 0], 0.3 / 255.0)
                        nc.vector.scalar_tensor_tensor(
                            t0[:], src[:, :, :, 1], 0.59 / 255.0, t0[:],
                            mybir.AluOpType.mult, mybir.AluOpType.add)
                        nc.vector.scalar_tensor_tensor(
                            dst[:], src[:, :, :, 2], 0.11 / 255.0, t0[:],
                            mybir.AluOpType.mult, mybir.AluOpType.add)

                    dr = ph1.tile([128, NG, R], F32)
                    db = ph1.tile([128, NG, R], F32)
                    nc.vector.scalar_tensor_tensor(
                        dr[:], a32[:, :, :, 0], 1.0 / 255.0, ya[:],
                        mybir.AluOpType.mult, mybir.AluOpType.subtract)
                    nc.vector.scalar_tensor_tensor(
                        db[:], a32[:, :, :, 2], 1.0 / 255.0, ya[:],
                        mybir.AluOpType.mult, mybir.AluOpType.subtract)
                    nc.vector.tensor_sub(t1[:], g32[:, :, :, 0], a32[:, :, :, 0])
                    nc.scalar.activation(s_abs[:], t1[:], mybir.ActivationFunctionType.Abs)
                    for ch in (1, 2):
                        nc.vector.tensor_sub(t1[:], g32[:, :, :, ch], a32[:, :, :, ch])
                        nc.scalar.activation(t2[:], t1[:], mybir.ActivationFunctionType.Abs)
                        nc.vector.tensor_add(s_abs[:], s_abs[:], t2[:])
                    nc.vector.tensor_scalar(cmask[:], s_abs[:], 2.55, None, mybir.AluOpType.is_gt)
                    nc.vector.tensor_scalar(notc[:], s_abs[:], 2.55, None, mybir.AluOpType.is_le)

                    iA = ph1.tile([128, NG, R], F32)
                    qA = ph1.tile([128, NG, R], F32)
                    nc.vector.tensor_scalar_mul(t1[:], db[:], -0.27)
                    nc.vector.scalar_tensor_tensor(
                        iA[:], dr[:], 0.74, t1[:], mybir.AluOpType.mult, mybir.AluOpType.add)
                    nc.vector.tensor_scalar_mul(t1[:], db[:], 0.41)
                    nc.vector.scalar_tensor_tensor(
                        qA[:], dr[:], 0.48, t1[:], mybir.AluOpType.mult, mybir.AluOpType.add)
                    nc.vector.tensor_mul(iA[:], iA[:], cmask[:])
                    nc.vector.tensor_mul(qA[:], qA[:], cmask[:])

                    nc.vector.memset(b16[:], 0.0)
                    bview = b16[:, PADE : PADE + W2].rearrange(
                        "p (g r c) -> p g r c", g=NG, r=R, c=2)
                    nc.vector.tensor_copy(bview[:, :, 1 : R - 1, 0], inner(iA))
                    nc.vector.tensor_copy(bview[:, :, 1 : R - 1, 1], inner(qA))
                    nc.vector.memset(x2[:], 0.0)
                    nc.vector.tensor_copy(x2[:, 0, :], b16[:])

                # ---------------- affinity weights ----------------
                with (
                    tc.tile_pool(name="ph2", bufs=1) as ph2,
                    tc.tile_pool(name="pp0", bufs=1, space="PSUM") as pp0,
                ):
                    # sentineled luminance + partition-shifted planes
                    ys = ph2.tile([128, NG, R], F32)
                    yp = ph2.tile([128, NG, R], F32)
                    ym = ph2.tile([128, NG, R], F32)
                    nc.vector.tensor_add(ys[:], y32[:], vs16[:])
                    nc.vector.memset(yp[:], SENT)
                    nc.vector.memset(ym[:], SENT)
                    nc.sync.dma_start(yp[0:127], ys[1:128])
                    nc.gpsimd.dma_start(ym[1:128], ys[0:127])
                    ypl = {1: yp, 0: ys, -1: ym}

                    # 3x3 box sums of y and y^2 (separable: rows on DVE,
                    # columns via tridiagonal matmul on PE)
                    r3 = ph2.tile([128, NG, R], F32)
                    r3q = ph2.tile([128, NG, R], F32)
                    y2 = ph2.tile([128, NG, R], F32)
                    nc.scalar.square(y2[:], y32[:])
                    nc.vector.scalar_tensor_tensor(
                        inner(r3), y32[:, :, 0 : R - 2], 1.0, y32[:, :, 2 : R],
                        mybir.AluOpType.mult, mybir.AluOpType.add)
                    nc.vector.tensor_add(inner(r3), inner(r3), inner(y32))
                    nc.vector.scalar_tensor_tensor(
                        inner(r3q), y2[:, :, 0 : R - 2], 1.0, y2[:, :, 2 : R],
                        mybir.AluOpType.mult, mybir.AluOpType.add)
                    nc.vector.tensor_add(inner(r3q), inner(r3q), inner(y2))

                    S1 = ph2.tile([128, NG, R], F32)
                    S2 = ph2.tile([128, NG, R], F32)
                    nc.vector.memset(S1[:], 0.0)
                    nc.vector.memset(S2[:], 0.0)
                    sbank = [pp0.tile([128, 512], F32, name=f"sb{i}",
                                      tag=f"sb{i}") for i in range(8)]
                    mbox32 = ph2.tile([128, 128], F32)
                    nc.vector.tensor_copy(mbox32[:], mats[:, 3, :])
                    RIN = R - 2
                    for j, (srct, dstt) in enumerate(((r3, S1), (r3q, S2))):
                        for g in range(NG):
                            ps = sbank[(j * NG + g) % 8]
                            nc.tensor.matmul(
                                ps[:, :RIN], mbox32[:],
                                srct[:, g, 1 : R - 1],
                                start=True, stop=True)
                            nc.scalar.copy(dstt[:, g, 1 : R - 1],
                                           ps[:, :RIN])

                    # var = S2*rc - (S1*rc)^2 ; negivs = -1/max(0.6 var, 2e-6)
                    m = ph2.tile([128, NG, R], F32)
                    m2 = ph2.tile([128, NG, R], F32)
                    nc.vector.tensor_mul(inner(m), inner(S1), inner(rc16))
                    nc.scalar.square(inner(m2), inner(m))
                    var = S2
                    nc.vector.tensor_mul(inner(var), inner(S2), inner(rc16))
                    nc.vector.tensor_sub(inner(var), inner(var), inner(m2))
                    negivs = S1
                    nc.vector.tensor_scalar(
                        inner(negivs), inner(var), 2e-6 / 0.6, None,
                        mybir.AluOpType.max)
                    nc.vector.reciprocal(inner(negivs), inner(negivs))
                    nc.vector.tensor_scalar_mul(inner(negivs), inner(negivs),
                                                -1.0 / 0.6)

                    # per-tap masked exp weights (sentinel kills invalid taps)
                    def shifted(plane, dx):
                        return plane[:, :, 1 + dx : R - 1 + dx]

                    mk = [ph2.tile([128, NG, R], F16, name=f"mk{k}", tag=f"mk{k}")
                          for k in range(8)]
                    for k, (dx, dy) in enumerate(OFFSETS):
                        d = ph2.tile([128, NG, R], F32, tag="d", bufs=3)
                        e = ph2.tile([128, NG, R], F32, tag="e", bufs=3)
                        nc.vector.tensor_sub(inner(d), shifted(ypl[dy], dx), inner(y32))
                        nc.scalar.square(inner(d), inner(d))
                        nc.vector.tensor_mul(inner(e), inner(d), inner(negivs))
                        nc.scalar.activation(
                            inner(mk[k]), inner(e), mybir.ActivationFunctionType.Exp)

                    # wsum (DVE/Pool split), wnorm = notc/max(wsum,eps)
                    wsum = m
                    wsb = m2
                    nc.vector.tensor_add(inner(wsum), inner(mk[0]), inner(mk[1]))
                    nc.gpsimd.tensor_add(inner(wsb), inner(mk[4]), inner(mk[5]))
                    nc.vector.tensor_add(inner(wsum), inner(wsum), inner(mk[2]))
                    nc.gpsimd.tensor_add(inner(wsb), inner(wsb), inner(mk[6]))
                    nc.vector.tensor_add(inner(wsum), inner(wsum), inner(mk[3]))
                    nc.gpsimd.tensor_add(inner(wsb), inner(wsb), inner(mk[7]))
                    nc.vector.tensor_add(inner(wsum), inner(wsum), inner(wsb))
                    wnorm = var
                    nc.vector.tensor_scalar(
                        inner(wnorm), inner(wsum), 1e-30, None, mybir.AluOpType.max)
                    nc.vector.reciprocal(inner(wnorm), inner(wnorm))
                    nc.vector.tensor_mul(inner(wnorm), inner(wnorm), inner(notc))
                    # zero the weights of out-of-image columns (the shifted-
                    # frame recompute below would otherwise give them exp(0)=1)
                    vbin = y2  # dead, reuse
                    nc.vector.tensor_scalar(vbin[:], vs16[:], 0.5, None,
                                            mybir.AluOpType.is_le)
                    nc.vector.tensor_mul(inner(wnorm), inner(wnorm), inner(vbin))

                    # partition-shifted planes of negivs and wnorm; with these
                    # the pre-shifted weights wde_k[p] = w_k[p - dy] can be
                    # recomputed directly in the shifted frame -- no strided
                    # partition-shift DMAs of the weight tensors needed.
                    negP = r3    # dead; reuse
                    negM = r3q   # dead; reuse
                    wnP = m      # (wsum) dead; reuse
                    wnM = m2     # (wsb) dead; reuse
                    nc.vector.memset(negP[:], -1.0)
                    nc.vector.memset(negM[:], -1.0)
                    nc.vector.memset(wnP[:], 0.0)
                    nc.vector.memset(wnM[:], 0.0)
                    nc.sync.dma_start(negP[0:127], negivs[1:128])
                    nc.gpsimd.dma_start(negM[1:128], negivs[0:127])
                    nc.sync.dma_start(wnP[0:127], wnorm[1:128])
                    nc.gpsimd.dma_start(wnM[1:128], wnorm[0:127])

                    # finalize: wde_k = dup(w~_k) with the partition pre-shift
                    # folded into the computation (center frame for dy==0,
                    # shifted frames for dy=+-1)
                    for k, (dx, dy) in enumerate(OFFSETS):
                        nc.vector.memset(wde[k][:], 0.0)
                        wv = wde[k][:].rearrange("p (g r c) -> p g r c",
                                                 g=NG, r=R, c=2)
                        if dy == 0:
                            mks, wn = mk[k], wnorm
                        else:
                            # frame shifted by -dy: center luma/params come
                            # from the opposite-shift planes
                            ctr = ym if dy == 1 else yp
                            ngv = negM if dy == 1 else negP
                            wn = wnM if dy == 1 else wnP
                            d = ph2.tile([128, NG, R], F32, tag="d", bufs=3)
                            e = ph2.tile([128, NG, R], F32, tag="e", bufs=3)
                            nc.vector.tensor_sub(inner(d), shifted(ys, dx),
                                                 inner(ctr))
                            nc.scalar.square(inner(d), inner(d))
                            nc.vector.tensor_mul(inner(e), inner(d), inner(ngv))
                            mks = ph2.tile([128, NG, R], F16, tag="mks", bufs=2)
                            nc.scalar.activation(
                                inner(mks), inner(e),
                                mybir.ActivationFunctionType.Exp)
                        nc.vector.tensor_mul(wv[:, :, 1 : R - 1, 0],
                                             inner(mks), inner(wn))
                        nc.gpsimd.tensor_mul(wv[:, :, 1 : R - 1, 1],
                                             inner(mks), inner(wn))

            # ---------------- Chebyshev iterations ----------------
            # PE term order: dy=-1, dy=+1 taps (DVE), then dy=0 with the Pool
            # tap last so PE never stalls on Pool early in a group.
            korder = [k for k, (dx, dy) in enumerate(OFFSETS) if dy == -1]
            korder += [k for k, (dx, dy) in enumerate(OFFSETS) if dy == 1]
            korder += [k for k, (dx, dy) in enumerate(OFFSETS)
                       if dy == 0 and k != p.pool_tap]
            korder += [p.pool_tap]
            PRANGE = {0: (0, 127), -1: (0, 127), 1: (0, 128)}

            def xview(buf, a, b):
                return x2[:, buf, PADE + a : PADE + b].rearrange(
                    "p (g r c) -> p g r c", g=(b - a) // R2, r=R, c=2)

            with (
                tc.tile_pool(name="qp", bufs=1) as qp,
                tc.tile_pool(name="pp", bufs=1, space="PSUM") as pp,
            ):
                banks = [pp.tile([128, 512], F32, name=f"bank{i}", tag=f"bank{i}")
                         for i in range(8)]
                qtiles = []
                for si, (g0, g1) in enumerate(sets):
                    sw = (g1 - g0) * R2
                    row = []
                    for k in range(8):
                        qt = qp.tile([128, sw], F16, name=f"qt{si}_{k}",
                                     tag=f"qt{si}_{k}")
                        nc.vector.memset(qt[:], 0.0)
                        row.append(qt)
                    qtiles.append(row)

                set_of_g = {}
                for si, (g0, g1) in enumerate(sets):
                    for g in range(g0, g1):
                        set_of_g[g] = si

                for it in range(p.n_iters):
                    t = it + 1
                    a_t, c_t = sched[it]
                    src = it % 2
                    dst = 1 - src

                    # Act: preload c_t * b into banks 0..7 (groups 0..7);
                    # group 8 shares bank 0 and is preloaded after group 0's
                    # evacuation inside the per-group loop below.
                    def preload(g):
                        nc.scalar.mul(
                            banks[g % 8][:, :R2],
                            b16[:, PADE + g * R2 : PADE + (g + 1) * R2],
                            float(c_t))

                    for g in range(min(NG, 8)):
                        preload(g)

                    # tap multiplies (per set)
                    for si, (g0, g1) in enumerate(sets):
                        lo2, hi2 = g0 * R2, g1 * R2
                        for k in korder:
                            dx, dy = OFFSETS[k]
                            qt = qtiles[si][k]
                            pa, pb = PRANGE[dy]
                            eng = nc.gpsimd if k == p.pool_tap else nc.vector
                            eng.tensor_mul(
                                qt[pa:pb],
                                wde[k][pa:pb, lo2:hi2],
                                x2[pa:pb, src, PADE + lo2 + 2 * dx : PADE + hi2 + 2 * dx],
                            )

                    # per-group: 8 matmuls -> Act scaled evac -> add -> guards
                    for g in range(NG):
                        if g >= 8:
                            preload(g)
                        si = set_of_g[g]
                        g0 = sets[si][0]
                        ps = banks[g % 8]
                        qoff = (g - g0) * R2
                        for ti, k in enumerate(korder):
                            nc.tensor.matmul(
                                ps[:, :R2], mats[:, MAT_IDX[OFFSETS[k][1]], :],
                                qtiles[si][k][:, qoff : qoff + R2],
                                start=False, stop=(ti == len(korder) - 1))
                        pv = ps[:, :R2].rearrange("p (r c) -> p r c", r=R, c=2)
                        ta = qp.tile([128, R2], F16, tag=f"ta{g % 4}", bufs=2)
                        tav = ta[:].rearrange("p (r c) -> p r c", r=R, c=2)
                        nc.scalar.mul(tav[:, 1 : R - 1, :], pv[:, 1 : R - 1, :],
                                      float(a_t))
                        dvw = xview(dst, g * R2, (g + 1) * R2)[:, 0]
                        eng = nc.vector if g < 5 else nc.gpsimd
                        eng.tensor_add(dvw[:, 1 : R - 1, :], tav[:, 1 : R - 1, :],
                                       dvw[:, 1 : R - 1, :])
                        # guard refresh with the left neighbor group
                        if g >= 1:
                            lf = xview(dst, (g - 1) * R2, g * R2)[:, 0]
                            nc.sync.dma_start(dvw[0:1, 1 : R - 1, :],
                                              lf[126:127, 1 : R - 1, :])
                            nc.gpsimd.dma_start(lf[127:128, 1 : R - 1, :],
                                                dvw[1:2, 1 : R - 1, :])

                    # halo exchange every T steps (both buffers)
                    if t % T == 0 and t < p.n_iters:
                        xr = x2[:, :, PADE : PADE + W2].rearrange(
                            "p b (g r c) -> p b g r c", g=NG, r=R, c=2)
                        for b_ in range(2):
                            nc.sync.dma_start(
                                xbnd[:, 0, b_], xr[:, b_, :, T + 1 : 2 * T + 1, :])
                            nc.scalar.dma_start(
                                xbnd[:, 1, b_], xr[:, b_, :, RPC + 1 : RPC + T + 1, :])
                        nc.gpsimd.collective_compute(
                            "AllGather",
                            mybir.AluOpType.bypass,
                            replica_groups=[list(range(p.ncores))],
                            ins=[xbnd.opt()],
                            outs=[xgath.opt()],
                        )
                        for r in range(p.ncores):
                            (nc.sync if r % 2 == 0 else nc.scalar).dma_start(
                                xg_sb[:, r], xgath[r])
                        for side, ucol in ((1, 0), (0, 8)):
                            for b_ in range(2):
                                if side == 1:
                                    dst_v = xr[:, b_, :, 1 : T + 1, :]
                                else:
                                    dst_v = xr[:, b_, :, RPC + T + 1 : RPC + 2 * T + 1, :]
                                nc.vector.tensor_scalar_mul(
                                    dst_v, xg_sb[:, 0, side, b_],
                                    uhot[:, ucol : ucol + 1])
                                for r in range(1, p.ncores):
                                    nc.vector.scalar_tensor_tensor(
                                        dst_v, xg_sb[:, r, side, b_],
                                        uhot[:, ucol + r : ucol + r + 1], dst_v,
                                        mybir.AluOpType.mult, mybir.AluOpType.add)

            # -------------- output: yiq2rgb on owned rows (fp16) --------------
            with tc.tile_pool(name="ph3", bufs=1) as ph3:
                o16 = ph3.tile([128, NG, RPC, 3], F16)
                y255 = ph3.tile([128, NG, RPC], F16)
                t3a = ph3.tile([128, NG, RPC], F16)
                fbuf = p.n_iters % 2
                xv = x2[:, fbuf, PADE : PADE + W2].rearrange(
                    "p (g r c) -> p g r c", g=NG, r=R, c=2)
                xi = xv[:, :, T + 1 : T + 1 + RPC, 0]
                xq = xv[:, :, T + 1 : T + 1 + RPC, 1]
                yo = y32[:, :, T + 1 : T + 1 + RPC]
                nc.scalar.mul(y255[:], yo, 255.0)
                inv_s = 255.0 / s_final
                for ch in range(3):
                    cy, ci, cq = YIQ2RGB[ch]
                    nc.vector.scalar_tensor_tensor(
                        t3a[:], xi, ci * inv_s, y255[:],
                        mybir.AluOpType.mult, mybir.AluOpType.add)
                    nc.vector.scalar_tensor_tensor(
                        t3a[:], xq, cq * inv_s, t3a[:],
                        mybir.AluOpType.mult, mybir.AluOpType.add)
                    nc.vector.tensor_scalar(
                        o16[:, :, :, ch], t3a[:], 0.0, 255.0,
                        mybir.AluOpType.max, mybir.AluOpType.min)
                nc.sync.dma_start(out_d[:], o16[:])

    nc.compile()
    return nc


# ---------------------------------------------------------------------------
# host-side sharding / assembly
# ---------------------------------------------------------------------------

def host_inputs(p: Params, gray: np.ndarray, appx: np.ndarray):
    H, W, T, NG, R, RPC = p.H, p.W, p.T, p.NG, p.R, p.rpc
    colw = p.cpg * NG + 2
    rpad = T + 1

    def padimg(img):
        return np.pad(
            img.astype(np.float32),
            ((rpad, R), (1, colw - 1 - W), (0, 0)),
        )

    gpad = padimg(gray)
    apad = padimg(appx)
    vpad = np.pad(np.ones((H, W), np.float32), ((rpad, R), (1, colw - 1 - W)))

    # count over the 3x3 box (valid neighbors + center)
    from numpy.lib.stride_tricks import sliding_window_view
    vp2 = np.pad(vpad, 1)
    cnt = sliding_window_view(vp2, (3, 3)).sum(axis=(2, 3))
    rcount_full = (1.0 / np.maximum(cnt, 1.0)).astype(np.float32)
    vsent_full = ((1.0 - vpad) * SENT).astype(np.float16)

    M = np.zeros((4, 128, 128), np.float16)
    for pp_ in range(1, 127):
        M[0, pp_, pp_] = 1
        M[1, pp_ + 1, pp_] = 1
        M[2, pp_ - 1, pp_] = 1
        M[3, pp_ - 1, pp_] = 1
        M[3, pp_, pp_] = 1
        M[3, pp_ + 1, pp_] = 1

    in_maps = []
    for c in range(p.ncores):
        r0 = RPC * c
        gT = np.empty((128, NG, R, 3), np.float16)
        aT = np.empty((128, NG, R, 3), np.float16)
        rT = np.empty((128, NG, R), np.float32)
        vT = np.empty((128, NG, R), np.float16)
        for g in range(NG):
            c0 = p.cpg * g
            gT[:, g] = gpad[r0 : r0 + R, c0 : c0 + 128].transpose(1, 0, 2)
            aT[:, g] = apad[r0 : r0 + R, c0 : c0 + 128].transpose(1, 0, 2)
            rT[:, g] = rcount_full[r0 : r0 + R, c0 : c0 + 128].T
            vT[:, g] = vsent_full[r0 : r0 + R, c0 : c0 + 128].T
        uhot = np.zeros((128, 16), np.float32)
        uhot[:, (c - 1) % p.ncores] = 1
        uhot[:, 8 + (c + 1) % p.ncores] = 1
        in_maps.append({"gray": np.ascontiguousarray(gT),
                        "appx": np.ascontiguousarray(aT),
                        "rcount": np.ascontiguousarray(rT),
                        "vsent": np.ascontiguousarray(vT),
                        "mats": M, "uhot": uhot})
    return in_maps


def assemble(p: Params, results):
    img = np.zeros((p.H, p.W, 3), np.float32)
    for c in range(p.ncores):
        o = np.asarray(results[c]["out"]).astype(np.float32)
        r0 = p.rpc * c
        for g in range(p.NG):
            ncols = min(p.cpg, p.W - p.cpg * g)
            img[r0 : r0 + p.rpc, p.cpg * g : p.cpg * g + ncols] = (
                o[1 : 1 + ncols, g].transpose(1, 0, 2))
    return img


# ---------------------------------------------------------------------------
# entry point
# ---------------------------------------------------------------------------

_CACHE = {}


def _get_program(p: Params):
    if p not in _CACHE:
        _CACHE[p] = build(p)
    return _CACHE[p]


def kernel(gray_rgb: np.ndarray, appendix_rgb: np.ndarray) -> np.ndarray:
    from concourse.bass_utils import run_bass_kernel_spmd

    p = Params()
    nc = _get_program(p)
    in_maps = host_inputs(p, np.asarray(gray_rgb), np.asarray(appendix_rgb))
    res = run_bass_kernel_spmd(nc, in_maps, list(range(p.ncores)))
    return assemble(p, res.results)


# revision 5
# speedup vs baseline: 1.0786x; 1.0245x over previous
"""Trainium2 Bass kernel v3: colorization via Chebyshev-accelerated Jacobi.

v3 over v2:
  - n=20 steps (rho=0.975), T=10 -> a single halo exchange
  - per-group PSUM banks (292-col matmuls, 8 rotating banks): PE stays hot,
    evacuation is fine-grained (Act scaled-copy + DVE/Pool add per group)
  - Act PSUM preload of c_t*b -> only 8 matmul terms (taps), start=False
  - setup overhaul: contiguous partition-major input DMAs; host-precomputed
    1/count and sentinel mask (kills all valid-mask multiplies); 3x3 box
    stats via separable row-sum + one tridiagonal PE matmul; squares/exp on
    Act; weight finalize split DVE/Pool
  - finale in fp16, fp16 output DMA
"""
import sys

sys.path.insert(0, "/opt/trn_rl_repo")

from dataclasses import dataclass

import numpy as np

import concourse.bass as bass
import concourse.bacc as bacc
import concourse.mybir as mybir
from concourse import tile

F32 = mybir.dt.float32
F16 = mybir.dt.float16

OFFSETS = [(-1, -1), (-1, 0), (-1, 1), (0, -1), (0, 1), (1, -1), (1, 0), (1, 1)]
MAT_IDX = {0: 0, 1: 1, -1: 2}
SENT = 30000.0  # luminance sentinel marking out-of-image pixels

YIQ2RGB = [
    [1.0, 0.9468822170900693, 0.6235565819861433],
    [1.0, -0.27478764629897834, -0.6356910791873801],
    [1.0, -1.1085450346420322, 1.7090069284064666],
]


# per-step omegas for the 3-term iteration, optimized offline (Krylov/Gram
# fit of the degree-18 polynomial against W^100 e0 on the actual instance,
# then validated in a full fp16 pipeline simulation: rgb rel err 5.4e-3)
OMS_OPT18 = [
    1.404064, 1.797732, 2.502787, 1.422837, 1.709504, 1.319234, 1.672824,
    1.213824, 1.865266, 1.482599, 3.084687, 1.583229, 1.485692, 1.638472,
    1.052554, 2.153161, 1.981092,
]


@dataclass(frozen=True)
class Params:
    H: int = 1024
    W: int = 1024
    ncores: int = 8
    n_iters: int = 20   # iteration steps (n=18+OMS_OPT18 is faster but its
                        # aggressively-fit polynomial is too sensitive to the
                        # HW affinity perturbation: 1.25e-2 measured vs the
                        # robust chebyshev n=20's 6.3e-3)
    rho: float = 0.975  # fallback chebyshev rho (used when OMS is None)
    T: int = 10         # ghost depth (iterations between halo exchanges)
    cpg: int = 126      # owned columns per partition-group
    ns: int = 2         # column-group sets for the tap multiplies
    act_preload: bool = True
    pool_tap: int = 6   # tap computed on Pool (dy==0 so it is last on PE)

    @property
    def rpc(self):
        return self.H // self.ncores

    @property
    def R(self):
        return self.rpc + 2 * self.T + 2

    @property
    def NG(self):
        return -(-self.W // self.cpg)

    @property
    def R2(self):
        return 2 * self.R

    @property
    def W2(self):
        return self.NG * self.R2


PADE = 4


def cheb_schedule(p: Params):
    if p.n_iters == len(OMS_OPT18) + 1:
        oms = list(OMS_OPT18)
    else:
        oms, om = [], 1.0
        for _t in range(2, p.n_iters + 1):
            om = 1.0 / (1.0 - 0.25 * p.rho * p.rho * om)
            oms.append(om)
    s_prev, s_cur = 1.0, 1.0
    sched = [(1.0, 1.0)]
    for om in oms:
        s_new = s_prev / (1.0 - om)
        a = s_new * om / s_cur
        c = s_cur
        sched.append((a, c))
        s_prev, s_cur = s_cur, s_new
    return sched, s_cur


def _sets(p: Params):
    base = p.NG // p.ns
    rem = p.NG % p.ns
    out, g0 = [], 0
    for s in range(p.ns):
        g1 = g0 + base + (1 if s < rem else 0)
        out.append((g0, g1))
        g0 = g1
    return out


def _chunks(width32: int, cap: int = 512):
    out, o = [], 0
    while o < width32:
        out.append((o, min(cap, width32 - o)))
        o += cap
    return out


def build(p: Params):
    nc = bacc.Bacc("TRN2", target_bir_lowering=False, debug=False,
                   num_devices=p.ncores)
    NG, R, R2, W2 = p.NG, p.R, p.R2, p.W2
    RPC, T = p.rpc, p.T
    W2P = W2 + 2 * PADE

    gray_d = nc.dram_tensor("gray", [128, NG, R, 3], F16, kind="ExternalInput")
    appx_d = nc.dram_tensor("appx", [128, NG, R, 3], F16, kind="ExternalInput")
    rcount_d = nc.dram_tensor("rcount", [128, NG, R], F32, kind="ExternalInput")
    vsent_d = nc.dram_tensor("vsent", [128, NG, R], F16, kind="ExternalInput")
    mats_d = nc.dram_tensor("mats", [4, 128, 128], F16, kind="ExternalInput")
    uhot_d = nc.dram_tensor("uhot", [128, 16], F32, kind="ExternalInput")
    out_d = nc.dram_tensor("out", [128, NG, RPC, 3], F16, kind="ExternalOutput")

    sets = _sets(p)
    sched, s_final = cheb_schedule(p)
    inner = lambda a: a[:, :, 1 : R - 1]

    with tile.TileContext(nc) as tc:
        with (
            tc.tile_pool(name="persist", bufs=1) as pers,
            tc.tile_pool(name="dram", bufs=1, space="DRAM") as dram,
        ):
            y32 = pers.tile([128, NG, R], F32)
            x2 = pers.tile([128, 2, W2P], F16)
            b16 = pers.tile([128, W2P], F16)
            wde = [pers.tile([128, W2], F16, name=f"wde{k}", tag=f"wde{k}")
                   for k in range(8)]
            mats = pers.tile([128, 4, 128], F16)
            uhot = pers.tile([128, 16], F32)
            xg_sb = pers.tile([128, p.ncores, 2, 2, NG, T, 2], F16)

            xbnd = dram.tile([128, 2, 2, NG, T, 2], F16)      # [side, buf, ...]
            xgath = dram.tile([p.ncores, 128, 2, 2, NG, T, 2], F16)

            for i in range(4):
                nc.scalar.dma_start(mats[:, i, :], mats_d[i])
            nc.scalar.dma_start(uhot[:], uhot_d[:])

            # ---------------- setup ----------------
            with tc.tile_pool(name="mid", bufs=1) as mid:
                notc = mid.tile([128, NG, R], F32)
                rc16 = mid.tile([128, NG, R], F32)
                vs16 = mid.tile([128, NG, R], F16)
                nc.scalar.dma_start(rc16[:], rcount_d[:])
                nc.scalar.dma_start(vs16[:], vsent_d[:])

                with tc.tile_pool(name="ph1", bufs=1) as ph1:
                    g32 = ph1.tile([128, NG, R, 3], F16)
                    a32 = ph1.tile([128, NG, R, 3], F16)
                    # chunked loads: the Act-issued hwdge queue spreads chunks
                    # across DMA engines; sync/gpsimd pin everything to DMA_0
                    qrr = [nc.scalar, nc.sync, nc.scalar, nc.gpsimd]
                    for g in range(NG):
                        qrr[g % 4].dma_start(g32[:, g], gray_d[:, g])
                        qrr[(g + 2) % 4].dma_start(a32[:, g], appx_d[:, g])

                    ya = ph1.tile([128, NG, R], F32)
                    t0 = ph1.tile([128, NG, R], F32)
                    t1 = ph1.tile([128, NG, R], F32)
                    t2 = ph1.tile([128, NG, R], F32)
                    s_abs = ph1.tile([128, NG, R], F32)
                    cmask = ph1.tile([128, NG, R], F32)

                    # y = (0.3 R + 0.59 G + 0.11 B)/255 for gray & appendix
                    for (src, dst) in ((g32, y32), (a32, ya)):
                        nc.vector.tensor_scalar_mul(t0[:], src[:, :, :, 0], 0.3 / 255.0)
                        nc.vector.scalar_tensor_tensor(
                            t0[:], src[:, :, :, 1], 0.59 / 255.0, t0[:],
                            mybir.AluOpType.mult, mybir.AluOpType.add)
                        nc.vector.scalar_tensor_tensor(
                            dst[:], src[:, :, :, 2], 0.11 / 255.0, t0[:],
                            mybir.AluOpType.mult, mybir.AluOpType.add)

                    dr = ph1.tile([128, NG, R], F32)
                    db = ph1.tile([128, NG, R], F32)
                    nc.vector.scalar_tensor_tensor(
                        dr[:], a32[:, :, :, 0], 1.0 / 255.0, ya[:],
                        mybir.AluOpType.mult, mybir.AluOpType.subtract)
                    nc.vector.scalar_tensor_tensor(
                        db[:], a32[:, :, :, 2], 1.0 / 255.0, ya[:],
                        mybir.AluOpType.mult, mybir.AluOpType.subtract)
                    nc.vector.tensor_sub(t1[:], g32[:, :, :, 0], a32[:, :, :, 0])
                    nc.scalar.activation(s_abs[:], t1[:], mybir.ActivationFunctionType.Abs)
                    for ch in (1, 2):
                        nc.vector.tensor_sub(t1[:], g32[:, :, :, ch], a32[:, :, :, ch])
                        nc.scalar.activation(t2[:], t1[:], mybir.ActivationFunctionType.Abs)
                        nc.vector.tensor_add(s_abs[:], s_abs[:], t2[:])
                    nc.vector.tensor_scalar(cmask[:], s_abs[:], 2.55, None, mybir.AluOpType.is_gt)
                    nc.vector.tensor_scalar(notc[:], s_abs[:], 2.55, None, mybir.AluOpType.is_le)

                    iA = ph1.tile([128, NG, R], F32)
                    qA = ph1.tile([128, NG, R], F32)
                    nc.vector.tensor_scalar_mul(t1[:], db[:], -0.27)
                    nc.vector.scalar_tensor_tensor(
                        iA[:], dr[:], 0.74, t1[:], mybir.AluOpType.mult, mybir.AluOpType.add)
                    nc.vector.tensor_scalar_mul(t1[:], db[:], 0.41)
                    nc.vector.scalar_tensor_tensor(
                        qA[:], dr[:], 0.48, t1[:], mybir.AluOpType.mult, mybir.AluOpType.add)
                    nc.vector.tensor_mul(iA[:], iA[:], cmask[:])
                    nc.vector.tensor_mul(qA[:], qA[:], cmask[:])

                    nc.vector.memset(b16[:], 0.0)
                    bview = b16[:, PADE : PADE + W2].rearrange(
                        "p (g r c) -> p g r c", g=NG, r=R, c=2)
                    nc.vector.tensor_copy(bview[:, :, 1 : R - 1, 0], inner(iA))
                    nc.vector.tensor_copy(bview[:, :, 1 : R - 1, 1], inner(qA))
                    nc.vector.memset(x2[:], 0.0)
                    nc.vector.tensor_copy(x2[:, 0, :], b16[:])

                # ---------------- affinity weights ----------------
                with (
                    tc.tile_pool(name="ph2", bufs=1) as ph2,
                    tc.tile_pool(name="pp0", bufs=1, space="PSUM") as pp0,
                ):
                    # sentineled luminance + partition-shifted planes
                    ys = ph2.tile([128, NG, R], F32)
                    yp = ph2.tile([128, NG, R], F32)
                    ym = ph2.tile([128, NG, R], F32)
                    nc.vector.tensor_add(ys[:], y32[:], vs16[:])
                    nc.vector.memset(yp[:], SENT)
                    nc.vector.memset(ym[:], SENT)
                    nc.sync.dma_start(yp[0:127], ys[1:128])
                    nc.gpsimd.dma_start(ym[1:128], ys[0:127])
                    ypl = {1: yp, 0: ys, -1: ym}

                    # 3x3 box sums of y and y^2 (separable: rows on DVE,
                    # columns via tridiagonal matmul on PE)
                    r3 = ph2.tile([128, NG, R], F32)
                    r3q = ph2.tile([128, NG, R], F32)
                    y2 = ph2.tile([128, NG, R], F32)
                    nc.scalar.square(y2[:], y32[:])
                    nc.vector.scalar_tensor_tensor(
                        inner(r3), y32[:, :, 0 : R - 2], 1.0, y32[:, :, 2 : R],
                        mybir.AluOpType.mult, mybir.AluOpType.add)
                    nc.vector.tensor_add(inner(r3), inner(r3), inner(y32))
                    nc.vector.scalar_tensor_tensor(
                        inner(r3q), y2[:, :, 0 : R - 2], 1.0, y2[:, :, 2 : R],
                        mybir.AluOpType.mult, mybir.AluOpType.add)
                    nc.vector.tensor_add(inner(r3q), inner(r3q), inner(y2))

                    S1 = ph2.tile([128, NG, R], F32)
                    S2 = ph2.tile([128, NG, R], F32)
                    nc.vector.memset(S1[:], 0.0)
                    nc.vector.memset(S2[:], 0.0)
                    sbank = [pp0.tile([128, 512], F32, name=f"sb{i}",
                                      tag=f"sb{i}") for i in range(8)]
                    mbox32 = ph2.tile([128, 128], F32)
                    nc.vector.tensor_copy(mbox32[:], mats[:, 3, :])
                    RIN = R - 2
                    for j, (srct, dstt) in enumerate(((r3, S1), (r3q, S2))):
                        for g in range(NG):
                            ps = sbank[(j * NG + g) % 8]
                            nc.tensor.matmul(
                                ps[:, :RIN], mbox32[:],
                                srct[:, g, 1 : R - 1],
                                start=True, stop=True)
                            nc.scalar.copy(dstt[:, g, 1 : R - 1],
                                           ps[:, :RIN])

                    # var = S2*rc - (S1*rc)^2 ; negivs = -1/max(0.6 var, 2e-6)
                    m = ph2.tile([128, NG, R], F32)
                    m2 = ph2.tile([128, NG, R], F32)
                    nc.vector.tensor_mul(inner(m), inner(S1), inner(rc16))
                    nc.scalar.square(inner(m2), inner(m))
                    var = S2
                    nc.vector.tensor_mul(inner(var), inner(S2), inner(rc16))
                    nc.vector.tensor_sub(inner(var), inner(var), inner(m2))
                    negivs = S1
                    nc.vector.tensor_scalar(
                        inner(negivs), inner(var), 2e-6 / 0.6, None,
                        mybir.AluOpType.max)
                    nc.vector.reciprocal(inner(negivs), inner(negivs))
                    nc.vector.tensor_scalar_mul(inner(negivs), inner(negivs),
                                                -1.0 / 0.6)

                    # per-tap masked exp weights (sentinel kills invalid taps)
                    def shifted(plane, dx):
                        return plane[:, :, 1 + dx : R - 1 + dx]

                    mk = [ph2.tile([128, NG, R], F16, name=f"mk{k}", tag=f"mk{k}")
                          for k in range(8)]
                    for k, (dx, dy) in enumerate(OFFSETS):
                        d = ph2.tile([128, NG, R], F32, tag="d", bufs=3)
                        e = ph2.tile([128, NG, R], F32, tag="e", bufs=3)
                        nc.vector.tensor_sub(inner(d), shifted(ypl[dy], dx), inner(y32))
                        nc.scalar.square(inner(d), inner(d))
                        nc.vector.tensor_mul(inner(e), inner(d), inner(negivs))
                        nc.scalar.activation(
                            inner(mk[k]), inner(e), mybir.ActivationFunctionType.Exp)

                    # wsum (DVE/Pool split), wnorm = notc/max(wsum,eps)
                    wsum = m
                    wsb = m2
                    nc.vector.tensor_add(inner(wsum), inner(mk[0]), inner(mk[1]))
                    nc.gpsimd.tensor_add(inner(wsb), inner(mk[4]), inner(mk[5]))
                    nc.vector.tensor_add(inner(wsum), inner(wsum), inner(mk[2]))
                    nc.gpsimd.tensor_add(inner(wsb), inner(wsb), inner(mk[6]))
                    nc.vector.tensor_add(inner(wsum), inner(wsum), inner(mk[3]))
                    nc.gpsimd.tensor_add(inner(wsb), inner(wsb), inner(mk[7]))
                    nc.vector.tensor_add(inner(wsum), inner(wsum), inner(wsb))
                    wnorm = var
                    nc.vector.tensor_scalar(
                        inner(wnorm), inner(wsum), 1e-30, None, mybir.AluOpType.max)
                    nc.vector.reciprocal(inner(wnorm), inner(wnorm))
                    nc.vector.tensor_mul(inner(wnorm), inner(wnorm), inner(notc))
                    # zero the weights of out-of-image columns (the shifted-
                    # frame recompute below would otherwise give them exp(0)=1)
                    vbin = y2  # dead, reuse
                    nc.vector.tensor_scalar(vbin[:], vs16[:], 0.5, None,
                                            mybir.AluOpType.is_le)
                    nc.vector.tensor_mul(inner(wnorm), inner(wnorm), inner(vbin))

                    # partition-shifted planes of negivs and wnorm; with these
                    # the pre-shifted weights wde_k[p] = w_k[p - dy] can be
                    # recomputed directly in the shifted frame -- no strided
                    # partition-shift DMAs of the weight tensors needed.
                    negP = r3    # dead; reuse
                    negM = r3q   # dead; reuse
                    wnP = m      # (wsum) dead; reuse
                    wnM = m2     # (wsb) dead; reuse
                    nc.vector.memset(negP[:], -1.0)
                    nc.vector.memset(negM[:], -1.0)
                    nc.vector.memset(wnP[:], 0.0)
                    nc.vector.memset(wnM[:], 0.0)
                    nc.sync.dma_start(negP[0:127], negivs[1:128])
                    nc.gpsimd.dma_start(negM[1:128], negivs[0:127])
                    nc.sync.dma_start(wnP[0:127], wnorm[1:128])
                    nc.gpsimd.dma_start(wnM[1:128], wnorm[0:127])

                    # finalize: wde_k = dup(w~_k) with the partition pre-shift
                    # folded into the computation (center frame for dy==0,
                    # shifted frames for dy=+-1)
                    for k, (dx, dy) in enumerate(OFFSETS):
                        nc.vector.memset(wde[k][:], 0.0)
                        wv = wde[k][:].rearrange("p (g r c) -> p g r c",
                                                 g=NG, r=R, c=2)
                        if dy == 0:
                            mks, wn = mk[k], wnorm
                        else:
                            # frame shifted by -dy: center luma/params come
                            # from the opposite-shift planes
                            ctr = ym if dy == 1 else yp
                            ngv = negM if dy == 1 else negP
                            wn = wnM if dy == 1 else wnP
                            d = ph2.tile([128, NG, R], F32, tag="d", bufs=3)
                            e = ph2.tile([128, NG, R], F32, tag="e", bufs=3)
                            nc.vector.tensor_sub(inner(d), shifted(ys, dx),
                                                 inner(ctr))
                            nc.scalar.square(inner(d), inner(d))
                            nc.vector.tensor_mul(inner(e), inner(d), inner(ngv))
                            mks = ph2.tile([128, NG, R], F16, tag="mks", bufs=2)
                            nc.scalar.activation(
                                inner(mks), inner(e),
                                mybir.ActivationFunctionType.Exp)
                        nc.vector.tensor_mul(wv[:, :, 1 : R - 1, 0],
                                             inner(mks), inner(wn))
                        nc.gpsimd.tensor_mul(wv[:, :, 1 : R - 1, 1],
                                             inner(mks), inner(wn))

            # ---------------- Chebyshev iterations ----------------
            # PE term order: dy=-1, dy=+1 taps (DVE), then dy=0 with the Pool
            # tap last so PE never stalls on Pool early in a group.
            korder = [k for k, (dx, dy) in enumerate(OFFSETS) if dy == -1]
            korder += [k for k, (dx, dy) in enumerate(OFFSETS) if dy == 1]
            korder += [k for k, (dx, dy) in enumerate(OFFSETS)
                       if dy == 0 and k != p.pool_tap]
            korder += [p.pool_tap]
            PRANGE = {0: (0, 127), -1: (0, 127), 1: (0, 128)}

            def xview(buf, a, b):
                return x2[:, buf, PADE + a : PADE + b].rearrange(
                    "p (g r c) -> p g r c", g=(b - a) // R2, r=R, c=2)

            with (
                tc.tile_pool(name="qp", bufs=1) as qp,
                tc.tile_pool(name="pp", bufs=1, space="PSUM") as pp,
            ):
                banks = [pp.tile([128, 512], F32, name=f"bank{i}", tag=f"bank{i}")
                         for i in range(8)]
                qtiles = []
                for si, (g0, g1) in enumerate(sets):
                    sw = (g1 - g0) * R2
                    row = []
                    for k in range(8):
                        qt = qp.tile([128, sw], F16, name=f"qt{si}_{k}",
                                     tag=f"qt{si}_{k}")
                        nc.vector.memset(qt[:], 0.0)
                        row.append(qt)
                    qtiles.append(row)

                set_of_g = {}
                for si, (g0, g1) in enumerate(sets):
                    for g in range(g0, g1):
                        set_of_g[g] = si

                for it in range(p.n_iters):
                    t = it + 1
                    a_t, c_t = sched[it]
                    src = it % 2
                    dst = 1 - src

                    # Act: preload c_t * b into banks 0..7 (groups 0..7);
                    # group 8 shares bank 0 and is preloaded after group 0's
                    # evacuation inside the per-group loop below.
                    def preload(g):
                        nc.scalar.mul(
                            banks[g % 8][:, :R2],
                            b16[:, PADE + g * R2 : PADE + (g + 1) * R2],
                            float(c_t))

                    for g in range(min(NG, 8)):
                        preload(g)

                    # tap multiplies (per set)
                    for si, (g0, g1) in enumerate(sets):
                        lo2, hi2 = g0 * R2, g1 * R2
                        for k in korder:
                            dx, dy = OFFSETS[k]
                            qt = qtiles[si][k]
                            pa, pb = PRANGE[dy]
                            eng = nc.gpsimd if k == p.pool_tap else nc.vector
                            eng.tensor_mul(
                                qt[pa:pb],
                                wde[k][pa:pb, lo2:hi2],
                                x2[pa:pb, src, PADE + lo2 + 2 * dx : PADE + hi2 + 2 * dx],
                            )

                    # per-group: 8 matmuls -> Act scaled evac -> add -> guards
                    for g in range(NG):
                        if g >= 8:
                            preload(g)
                        si = set_of_g[g]
                        g0 = sets[si][0]
                        ps = banks[g % 8]
                        qoff = (g - g0) * R2
                        for ti, k in enumerate(korder):
                            nc.tensor.matmul(
                                ps[:, :R2], mats[:, MAT_IDX[OFFSETS[k][1]], :],
                                qtiles[si][k][:, qoff : qoff + R2],
                                start=False, stop=(ti == len(korder) - 1))
                        # single-instruction evacuation: x^_new = a_t*PSUM +
                        # x^_prev in-place on DVE (shortest possible tail:
                        # no Act hop, no staging tile)
                        pv = ps[:, :R2].rearrange("p (r c) -> p r c", r=R, c=2)
                        dvw = xview(dst, g * R2, (g + 1) * R2)[:, 0]
                        nc.vector.scalar_tensor_tensor(
                            dvw[:, 1 : R - 1, :], pv[:, 1 : R - 1, :],
                            float(a_t), dvw[:, 1 : R - 1, :],
                            mybir.AluOpType.mult, mybir.AluOpType.add)
                        # guard refresh with the left neighbor group
                        if g >= 1:
                            lf = xview(dst, (g - 1) * R2, g * R2)[:, 0]
                            nc.sync.dma_start(dvw[0:1, 1 : R - 1, :],
                                              lf[126:127, 1 : R - 1, :])
                            nc.gpsimd.dma_start(lf[127:128, 1 : R - 1, :],
                                                dvw[1:2, 1 : R - 1, :])

                    # halo exchange every T steps (both buffers)
                    if t % T == 0 and t < p.n_iters:
                        xr = x2[:, :, PADE : PADE + W2].rearrange(
                            "p b (g r c) -> p b g r c", g=NG, r=R, c=2)
                        for b_ in range(2):
                            nc.sync.dma_start(
                                xbnd[:, 0, b_], xr[:, b_, :, T + 1 : 2 * T + 1, :])
                            nc.scalar.dma_start(
                                xbnd[:, 1, b_], xr[:, b_, :, RPC + 1 : RPC + T + 1, :])
                        nc.gpsimd.collective_compute(
                            "AllGather",
                            mybir.AluOpType.bypass,
                            replica_groups=[list(range(p.ncores))],
                            ins=[xbnd.opt()],
                            outs=[xgath.opt()],
                        )
                        for r in range(p.ncores):
                            (nc.sync if r % 2 == 0 else nc.scalar).dma_start(
                                xg_sb[:, r], xgath[r])
                        for side, ucol in ((1, 0), (0, 8)):
                            for b_ in range(2):
                                if side == 1:
                                    dst_v = xr[:, b_, :, 1 : T + 1, :]
                                else:
                                    dst_v = xr[:, b_, :, RPC + T + 1 : RPC + 2 * T + 1, :]
                                nc.vector.tensor_scalar_mul(
                                    dst_v, xg_sb[:, 0, side, b_],
                                    uhot[:, ucol : ucol + 1])
                                for r in range(1, p.ncores):
                                    nc.vector.scalar_tensor_tensor(
                                        dst_v, xg_sb[:, r, side, b_],
                                        uhot[:, ucol + r : ucol + r + 1], dst_v,
                                        mybir.AluOpType.mult, mybir.AluOpType.add)

            # -------------- output: yiq2rgb on owned rows (fp16) --------------
            with tc.tile_pool(name="ph3", bufs=1) as ph3:
                o16 = ph3.tile([128, NG, RPC, 3], F16)
                y255 = ph3.tile([128, NG, RPC], F16)
                t3a = ph3.tile([128, NG, RPC], F16)
                fbuf = p.n_iters % 2
                xv = x2[:, fbuf, PADE : PADE + W2].rearrange(
                    "p (g r c) -> p g r c", g=NG, r=R, c=2)
                xi = xv[:, :, T + 1 : T + 1 + RPC, 0]
                xq = xv[:, :, T + 1 : T + 1 + RPC, 1]
                yo = y32[:, :, T + 1 : T + 1 + RPC]
                nc.scalar.mul(y255[:], yo, 255.0)
                inv_s = 255.0 / s_final
                for ch in range(3):
                    cy, ci, cq = YIQ2RGB[ch]
                    nc.vector.scalar_tensor_tensor(
                        t3a[:], xi, ci * inv_s, y255[:],
                        mybir.AluOpType.mult, mybir.AluOpType.add)
                    nc.vector.scalar_tensor_tensor(
                        t3a[:], xq, cq * inv_s, t3a[:],
                        mybir.AluOpType.mult, mybir.AluOpType.add)
                    nc.vector.tensor_scalar(
                        o16[:, :, :, ch], t3a[:], 0.0, 255.0,
                        mybir.AluOpType.max, mybir.AluOpType.min)
                nc.sync.dma_start(out_d[:], o16[:])

    nc.compile()
    return nc


# ---------------------------------------------------------------------------
# host-side sharding / assembly
# ---------------------------------------------------------------------------

def host_inputs(p: Params, gray: np.ndarray, appx: np.ndarray):
    H, W, T, NG, R, RPC = p.H, p.W, p.T, p.NG, p.R, p.rpc
    colw = p.cpg * NG + 2
    rpad = T + 1

    def padimg(img):
        return np.pad(
            img.astype(np.float32),
            ((rpad, R), (1, colw - 1 - W), (0, 0)),
        )

    gpad = padimg(gray)
    apad = padimg(appx)
    vpad = np.pad(np.ones((H, W), np.float32), ((rpad, R), (1, colw - 1 - W)))

    # count over the 3x3 box (valid neighbors + center)
    from numpy.lib.stride_tricks import sliding_window_view
    vp2 = np.pad(vpad, 1)
    cnt = sliding_window_view(vp2, (3, 3)).sum(axis=(2, 3))
    rcount_full = (1.0 / np.maximum(cnt, 1.0)).astype(np.float32)
    vsent_full = ((1.0 - vpad) * SENT).astype(np.float16)

    M = np.zeros((4, 128, 128), np.float16)
    for pp_ in range(1, 127):
        M[0, pp_, pp_] = 1
        M[1, pp_ + 1, pp_] = 1
        M[2, pp_ - 1, pp_] = 1
        M[3, pp_ - 1, pp_] = 1
        M[3, pp_, pp_] = 1
        M[3, pp_ + 1, pp_] = 1

    in_maps = []
    for c in range(p.ncores):
        r0 = RPC * c
        gT = np.empty((128, NG, R, 3), np.float16)
        aT = np.empty((128, NG, R, 3), np.float16)
        rT = np.empty((128, NG, R), np.float32)
        vT = np.empty((128, NG, R), np.float16)
        for g in range(NG):
            c0 = p.cpg * g
            gT[:, g] = gpad[r0 : r0 + R, c0 : c0 + 128].transpose(1, 0, 2)
            aT[:, g] = apad[r0 : r0 + R, c0 : c0 + 128].transpose(1, 0, 2)
            rT[:, g] = rcount_full[r0 : r0 + R, c0 : c0 + 128].T
            vT[:, g] = vsent_full[r0 : r0 + R, c0 : c0 + 128].T
        uhot = np.zeros((128, 16), np.float32)
        uhot[:, (c - 1) % p.ncores] = 1
        uhot[:, 8 + (c + 1) % p.ncores] = 1
        in_maps.append({"gray": np.ascontiguousarray(gT),
                        "appx": np.ascontiguousarray(aT),
                        "rcount": np.ascontiguousarray(rT),
                        "vsent": np.ascontiguousarray(vT),
                        "mats": M, "uhot": uhot})
    return in_maps


def assemble(p: Params, results):
    img = np.zeros((p.H, p.W, 3), np.float32)
    for c in range(p.ncores):
        o = np.asarray(results[c]["out"]).astype(np.float32)
        r0 = p.rpc * c
        for g in range(p.NG):
            ncols = min(p.cpg, p.W - p.cpg * g)
            img[r0 : r0 + p.rpc, p.cpg * g : p.cpg * g + ncols] = (
                o[1 : 1 + ncols, g].transpose(1, 0, 2))
    return img


# ---------------------------------------------------------------------------
# entry point
# ---------------------------------------------------------------------------

_CACHE = {}


def _get_program(p: Params):
    if p not in _CACHE:
        _CACHE[p] = build(p)
    return _CACHE[p]


def kernel(gray_rgb: np.ndarray, appendix_rgb: np.ndarray) -> np.ndarray:
    from concourse.bass_utils import run_bass_kernel_spmd

    p = Params()
    nc = _get_program(p)
    in_maps = host_inputs(p, np.asarray(gray_rgb), np.asarray(appendix_rgb))
    res = run_bass_kernel_spmd(nc, in_maps, list(range(p.ncores)))
    return assemble(p, res.results)


# revision 6
# speedup vs baseline: 1.1707x; 1.0854x over previous
"""Trainium2 Bass kernel v3: colorization via Chebyshev-accelerated Jacobi.

v3 over v2:
  - n=20 steps (rho=0.975), T=10 -> a single halo exchange
  - per-group PSUM banks (292-col matmuls, 8 rotating banks): PE stays hot,
    evacuation is fine-grained (Act scaled-copy + DVE/Pool add per group)
  - Act PSUM preload of c_t*b -> only 8 matmul terms (taps), start=False
  - setup overhaul: contiguous partition-major input DMAs; host-precomputed
    1/count and sentinel mask (kills all valid-mask multiplies); 3x3 box
    stats via separable row-sum + one tridiagonal PE matmul; squares/exp on
    Act; weight finalize split DVE/Pool
  - finale in fp16, fp16 output DMA
"""
import sys

sys.path.insert(0, "/opt/trn_rl_repo")

from dataclasses import dataclass

import numpy as np

import concourse.bass as bass
import concourse.bacc as bacc
import concourse.mybir as mybir
from concourse import tile

F32 = mybir.dt.float32
F16 = mybir.dt.float16

OFFSETS = [(-1, -1), (-1, 0), (-1, 1), (0, -1), (0, 1), (1, -1), (1, 0), (1, 1)]
MAT_IDX = {0: 0, 1: 1, -1: 2}
SENT = 30000.0  # luminance sentinel marking out-of-image pixels

YIQ2RGB = [
    [1.0, 0.9468822170900693, 0.6235565819861433],
    [1.0, -0.27478764629897834, -0.6356910791873801],
    [1.0, -1.1085450346420322, 1.7090069284064666],
]


# per-step omegas for the 3-term iteration, optimized offline (Krylov/Gram
# fit of the degree-18 polynomial against W^100 e0 on the actual instance,
# then validated in a full fp16 pipeline simulation: rgb rel err 5.4e-3)
OMS_OPT18 = [
    1.404064, 1.797732, 2.502787, 1.422837, 1.709504, 1.319234, 1.672824,
    1.213824, 1.865266, 1.482599, 3.084687, 1.583229, 1.485692, 1.638472,
    1.052554, 2.153161, 1.981092,
]


@dataclass(frozen=True)
class Params:
    H: int = 1024
    W: int = 1024
    ncores: int = 8
    n_iters: int = 19   # iteration steps (n=18+OMS_OPT18 is faster but its
                        # aggressively-fit polynomial is too sensitive to the
                        # HW affinity perturbation: 1.25e-2 measured vs the
                        # robust chebyshev n=20's 6.3e-3)
    rho: float = 0.975  # fallback chebyshev rho (used when OMS is None)
    T: int = 10         # ghost depth (iterations between halo exchanges)
    cpg: int = 126      # owned columns per partition-group
    ns: int = 2         # column-group sets for the tap multiplies
    act_preload: bool = True
    pool_tap: int = 6   # tap computed on Pool (dy==0 so it is last on PE)

    @property
    def rpc(self):
        return self.H // self.ncores

    @property
    def R(self):
        return self.rpc + 2 * self.T + 2

    @property
    def NG(self):
        return -(-self.W // self.cpg)

    @property
    def R2(self):
        return 2 * self.R

    @property
    def W2(self):
        return self.NG * self.R2


PADE = 4


def cheb_schedule(p: Params):
    if p.n_iters == len(OMS_OPT18) + 1:
        oms = list(OMS_OPT18)
    else:
        oms, om = [], 1.0
        for _t in range(2, p.n_iters + 1):
            om = 1.0 / (1.0 - 0.25 * p.rho * p.rho * om)
            oms.append(om)
    s_prev, s_cur = 1.0, 1.0
    sched = [(1.0, 1.0)]
    for om in oms:
        s_new = s_prev / (1.0 - om)
        a = s_new * om / s_cur
        c = s_cur
        sched.append((a, c))
        s_prev, s_cur = s_cur, s_new
    return sched, s_cur


def _sets(p: Params):
    base = p.NG // p.ns
    rem = p.NG % p.ns
    out, g0 = [], 0
    for s in range(p.ns):
        g1 = g0 + base + (1 if s < rem else 0)
        out.append((g0, g1))
        g0 = g1
    return out


def _chunks(width32: int, cap: int = 512):
    out, o = [], 0
    while o < width32:
        out.append((o, min(cap, width32 - o)))
        o += cap
    return out


def build(p: Params):
    nc = bacc.Bacc("TRN2", target_bir_lowering=False, debug=False,
                   num_devices=p.ncores)
    NG, R, R2, W2 = p.NG, p.R, p.R2, p.W2
    RPC, T = p.rpc, p.T
    W2P = W2 + 2 * PADE

    gray_d = nc.dram_tensor("gray", [128, NG, R, 3], F16, kind="ExternalInput")
    appx_d = nc.dram_tensor("appx", [128, NG, R, 3], F16, kind="ExternalInput")
    rcount_d = nc.dram_tensor("rcount", [128, NG, R], F32, kind="ExternalInput")
    vsent_d = nc.dram_tensor("vsent", [128, NG, R], F16, kind="ExternalInput")
    mats_d = nc.dram_tensor("mats", [4, 128, 128], F16, kind="ExternalInput")
    uhot_d = nc.dram_tensor("uhot", [128, 16], F32, kind="ExternalInput")
    out_d = nc.dram_tensor("out", [128, NG, RPC, 3], F16, kind="ExternalOutput")

    sets = _sets(p)
    sched, s_final = cheb_schedule(p)
    inner = lambda a: a[:, :, 1 : R - 1]

    with tile.TileContext(nc) as tc:
        with (
            tc.tile_pool(name="persist", bufs=1) as pers,
            tc.tile_pool(name="dram", bufs=1, space="DRAM") as dram,
        ):
            y32 = pers.tile([128, NG, R], F32)
            x2 = pers.tile([128, 2, W2P], F16)
            b16 = pers.tile([128, W2P], F16)
            wde = [pers.tile([128, W2], F16, name=f"wde{k}", tag=f"wde{k}")
                   for k in range(8)]
            mats = pers.tile([128, 4, 128], F16)
            uhot = pers.tile([128, 16], F32)
            xg_sb = pers.tile([128, p.ncores, 2, 2, NG, T, 2], F16)

            xbnd = dram.tile([128, 2, 2, NG, T, 2], F16)      # [side, buf, ...]
            xgath = dram.tile([p.ncores, 128, 2, 2, NG, T, 2], F16)

            for i in range(4):
                nc.scalar.dma_start(mats[:, i, :], mats_d[i])
            nc.scalar.dma_start(uhot[:], uhot_d[:])

            # ---------------- setup ----------------
            with tc.tile_pool(name="mid", bufs=1) as mid:
                notc = mid.tile([128, NG, R], F32)
                rc16 = mid.tile([128, NG, R], F32)
                vs16 = mid.tile([128, NG, R], F16)
                nc.scalar.dma_start(rc16[:], rcount_d[:])
                nc.scalar.dma_start(vs16[:], vsent_d[:])

                with tc.tile_pool(name="ph1", bufs=1) as ph1:
                    g32 = ph1.tile([128, NG, R, 3], F16)
                    a32 = ph1.tile([128, NG, R, 3], F16)
                    # chunked loads: the Act-issued hwdge queue spreads chunks
                    # across DMA engines; sync/gpsimd pin everything to DMA_0
                    qrr = [nc.scalar, nc.sync, nc.scalar, nc.gpsimd]
                    for g in range(NG):
                        qrr[g % 4].dma_start(g32[:, g], gray_d[:, g])
                        qrr[(g + 2) % 4].dma_start(a32[:, g], appx_d[:, g])

                    ya = ph1.tile([128, NG, R], F32)
                    t0 = ph1.tile([128, NG, R], F32)
                    t1 = ph1.tile([128, NG, R], F32)
                    t2 = ph1.tile([128, NG, R], F32)
                    s_abs = ph1.tile([128, NG, R], F32)
                    cmask = ph1.tile([128, NG, R], F32)

                    # y = (0.3 R + 0.59 G + 0.11 B)/255 for gray & appendix
                    for (src, dst) in ((g32, y32), (a32, ya)):
                        nc.vector.tensor_scalar_mul(t0[:], src[:, :, :, 0], 0.3 / 255.0)
                        nc.vector.scalar_tensor_tensor(
                            t0[:], src[:, :, :, 1], 0.59 / 255.0, t0[:],
                            mybir.AluOpType.mult, mybir.AluOpType.add)
                        nc.vector.scalar_tensor_tensor(
                            dst[:], src[:, :, :, 2], 0.11 / 255.0, t0[:],
                            mybir.AluOpType.mult, mybir.AluOpType.add)

                    dr = ph1.tile([128, NG, R], F32)
                    db = ph1.tile([128, NG, R], F32)
                    nc.vector.scalar_tensor_tensor(
                        dr[:], a32[:, :, :, 0], 1.0 / 255.0, ya[:],
                        mybir.AluOpType.mult, mybir.AluOpType.subtract)
                    nc.vector.scalar_tensor_tensor(
                        db[:], a32[:, :, :, 2], 1.0 / 255.0, ya[:],
                        mybir.AluOpType.mult, mybir.AluOpType.subtract)
                    nc.vector.tensor_sub(t1[:], g32[:, :, :, 0], a32[:, :, :, 0])
                    nc.scalar.activation(s_abs[:], t1[:], mybir.ActivationFunctionType.Abs)
                    for ch in (1, 2):
                        nc.vector.tensor_sub(t1[:], g32[:, :, :, ch], a32[:, :, :, ch])
                        nc.scalar.activation(t2[:], t1[:], mybir.ActivationFunctionType.Abs)
                        nc.vector.tensor_add(s_abs[:], s_abs[:], t2[:])
                    nc.vector.tensor_scalar(cmask[:], s_abs[:], 2.55, None, mybir.AluOpType.is_gt)
                    nc.vector.tensor_scalar(notc[:], s_abs[:], 2.55, None, mybir.AluOpType.is_le)

                    iA = ph1.tile([128, NG, R], F32)
                    qA = ph1.tile([128, NG, R], F32)
                    nc.vector.tensor_scalar_mul(t1[:], db[:], -0.27)
                    nc.vector.scalar_tensor_tensor(
                        iA[:], dr[:], 0.74, t1[:], mybir.AluOpType.mult, mybir.AluOpType.add)
                    nc.vector.tensor_scalar_mul(t1[:], db[:], 0.41)
                    nc.vector.scalar_tensor_tensor(
                        qA[:], dr[:], 0.48, t1[:], mybir.AluOpType.mult, mybir.AluOpType.add)
                    nc.vector.tensor_mul(iA[:], iA[:], cmask[:])
                    nc.vector.tensor_mul(qA[:], qA[:], cmask[:])

                    nc.vector.memset(b16[:], 0.0)
                    bview = b16[:, PADE : PADE + W2].rearrange(
                        "p (g r c) -> p g r c", g=NG, r=R, c=2)
                    nc.vector.tensor_copy(bview[:, :, 1 : R - 1, 0], inner(iA))
                    nc.vector.tensor_copy(bview[:, :, 1 : R - 1, 1], inner(qA))
                    nc.vector.memset(x2[:], 0.0)
                    nc.vector.tensor_copy(x2[:, 0, :], b16[:])

                # ---------------- affinity weights ----------------
                with (
                    tc.tile_pool(name="ph2", bufs=1) as ph2,
                    tc.tile_pool(name="pp0", bufs=1, space="PSUM") as pp0,
                ):
                    # sentineled luminance + partition-shifted planes
                    ys = ph2.tile([128, NG, R], F32)
                    yp = ph2.tile([128, NG, R], F32)
                    ym = ph2.tile([128, NG, R], F32)
                    nc.vector.tensor_add(ys[:], y32[:], vs16[:])
                    nc.vector.memset(yp[:], SENT)
                    nc.vector.memset(ym[:], SENT)
                    nc.sync.dma_start(yp[0:127], ys[1:128])
                    nc.gpsimd.dma_start(ym[1:128], ys[0:127])
                    ypl = {1: yp, 0: ys, -1: ym}

                    # 3x3 box sums of y and y^2 (separable: rows on DVE,
                    # columns via tridiagonal matmul on PE)
                    r3 = ph2.tile([128, NG, R], F32)
                    r3q = ph2.tile([128, NG, R], F32)
                    y2 = ph2.tile([128, NG, R], F32)
                    nc.scalar.square(y2[:], y32[:])
                    nc.vector.scalar_tensor_tensor(
                        inner(r3), y32[:, :, 0 : R - 2], 1.0, y32[:, :, 2 : R],
                        mybir.AluOpType.mult, mybir.AluOpType.add)
                    nc.vector.tensor_add(inner(r3), inner(r3), inner(y32))
                    nc.vector.scalar_tensor_tensor(
                        inner(r3q), y2[:, :, 0 : R - 2], 1.0, y2[:, :, 2 : R],
                        mybir.AluOpType.mult, mybir.AluOpType.add)
                    nc.vector.tensor_add(inner(r3q), inner(r3q), inner(y2))

                    S1 = ph2.tile([128, NG, R], F32)
                    S2 = ph2.tile([128, NG, R], F32)
                    nc.vector.memset(S1[:], 0.0)
                    nc.vector.memset(S2[:], 0.0)
                    sbank = [pp0.tile([128, 512], F32, name=f"sb{i}",
                                      tag=f"sb{i}") for i in range(8)]
                    mbox32 = ph2.tile([128, 128], F32)
                    nc.vector.tensor_copy(mbox32[:], mats[:, 3, :])
                    RIN = R - 2
                    for j, (srct, dstt) in enumerate(((r3, S1), (r3q, S2))):
                        for g in range(NG):
                            ps = sbank[(j * NG + g) % 8]
                            nc.tensor.matmul(
                                ps[:, :RIN], mbox32[:],
                                srct[:, g, 1 : R - 1],
                                start=True, stop=True)
                            nc.scalar.copy(dstt[:, g, 1 : R - 1],
                                           ps[:, :RIN])

                    # var = S2*rc - (S1*rc)^2 ; negivs = -1/max(0.6 var, 2e-6)
                    m = ph2.tile([128, NG, R], F32)
                    m2 = ph2.tile([128, NG, R], F32)
                    nc.vector.tensor_mul(inner(m), inner(S1), inner(rc16))
                    nc.scalar.square(inner(m2), inner(m))
                    var = S2
                    nc.vector.tensor_mul(inner(var), inner(S2), inner(rc16))
                    nc.vector.tensor_sub(inner(var), inner(var), inner(m2))
                    negivs = S1
                    nc.vector.tensor_scalar(
                        inner(negivs), inner(var), 2e-6 / 0.6, None,
                        mybir.AluOpType.max)
                    nc.vector.reciprocal(inner(negivs), inner(negivs))
                    nc.vector.tensor_scalar_mul(inner(negivs), inner(negivs),
                                                -1.0 / 0.6)

                    # per-tap masked exp weights (sentinel kills invalid taps)
                    def shifted(plane, dx):
                        return plane[:, :, 1 + dx : R - 1 + dx]

                    mk = [ph2.tile([128, NG, R], F16, name=f"mk{k}", tag=f"mk{k}")
                          for k in range(8)]
                    for k, (dx, dy) in enumerate(OFFSETS):
                        d = ph2.tile([128, NG, R], F32, tag="d", bufs=3)
                        e = ph2.tile([128, NG, R], F32, tag="e", bufs=3)
                        nc.vector.tensor_sub(inner(d), shifted(ypl[dy], dx), inner(y32))
                        nc.scalar.square(inner(d), inner(d))
                        nc.vector.tensor_mul(inner(e), inner(d), inner(negivs))
                        nc.scalar.activation(
                            inner(mk[k]), inner(e), mybir.ActivationFunctionType.Exp)

                    # wsum (DVE/Pool split), wnorm = notc/max(wsum,eps)
                    wsum = m
                    wsb = m2
                    nc.vector.tensor_add(inner(wsum), inner(mk[0]), inner(mk[1]))
                    nc.gpsimd.tensor_add(inner(wsb), inner(mk[4]), inner(mk[5]))
                    nc.vector.tensor_add(inner(wsum), inner(wsum), inner(mk[2]))
                    nc.gpsimd.tensor_add(inner(wsb), inner(wsb), inner(mk[6]))
                    nc.vector.tensor_add(inner(wsum), inner(wsum), inner(mk[3]))
                    nc.gpsimd.tensor_add(inner(wsb), inner(wsb), inner(mk[7]))
                    nc.vector.tensor_add(inner(wsum), inner(wsum), inner(wsb))
                    wnorm = var
                    nc.vector.tensor_scalar(
                        inner(wnorm), inner(wsum), 1e-30, None, mybir.AluOpType.max)
                    nc.vector.reciprocal(inner(wnorm), inner(wnorm))
                    nc.vector.tensor_mul(inner(wnorm), inner(wnorm), inner(notc))
                    # zero the weights of out-of-image columns (the shifted-
                    # frame recompute below would otherwise give them exp(0)=1)
                    vbin = y2  # dead, reuse
                    nc.vector.tensor_scalar(vbin[:], vs16[:], 0.5, None,
                                            mybir.AluOpType.is_le)
                    nc.vector.tensor_mul(inner(wnorm), inner(wnorm), inner(vbin))

                    # partition-shifted planes of negivs and wnorm; with these
                    # the pre-shifted weights wde_k[p] = w_k[p - dy] can be
                    # recomputed directly in the shifted frame -- no strided
                    # partition-shift DMAs of the weight tensors needed.
                    negP = r3    # dead; reuse
                    negM = r3q   # dead; reuse
                    wnP = m      # (wsum) dead; reuse
                    wnM = m2     # (wsb) dead; reuse
                    nc.vector.memset(negP[:], -1.0)
                    nc.vector.memset(negM[:], -1.0)
                    nc.vector.memset(wnP[:], 0.0)
                    nc.vector.memset(wnM[:], 0.0)
                    nc.sync.dma_start(negP[0:127], negivs[1:128])
                    nc.gpsimd.dma_start(negM[1:128], negivs[0:127])
                    nc.sync.dma_start(wnP[0:127], wnorm[1:128])
                    nc.gpsimd.dma_start(wnM[1:128], wnorm[0:127])

                    # finalize: wde_k = dup(w~_k) with the partition pre-shift
                    # folded into the computation (center frame for dy==0,
                    # shifted frames for dy=+-1)
                    for k, (dx, dy) in enumerate(OFFSETS):
                        nc.vector.memset(wde[k][:], 0.0)
                        wv = wde[k][:].rearrange("p (g r c) -> p g r c",
                                                 g=NG, r=R, c=2)
                        if dy == 0:
                            mks, wn = mk[k], wnorm
                        else:
                            # frame shifted by -dy: center luma/params come
                            # from the opposite-shift planes
                            ctr = ym if dy == 1 else yp
                            ngv = negM if dy == 1 else negP
                            wn = wnM if dy == 1 else wnP
                            d = ph2.tile([128, NG, R], F32, tag="d", bufs=3)
                            e = ph2.tile([128, NG, R], F32, tag="e", bufs=3)
                            nc.vector.tensor_sub(inner(d), shifted(ys, dx),
                                                 inner(ctr))
                            nc.scalar.square(inner(d), inner(d))
                            nc.vector.tensor_mul(inner(e), inner(d), inner(ngv))
                            mks = ph2.tile([128, NG, R], F16, tag="mks", bufs=2)
                            nc.scalar.activation(
                                inner(mks), inner(e),
                                mybir.ActivationFunctionType.Exp)
                        nc.vector.tensor_mul(wv[:, :, 1 : R - 1, 0],
                                             inner(mks), inner(wn))
                        nc.gpsimd.tensor_mul(wv[:, :, 1 : R - 1, 1],
                                             inner(mks), inner(wn))

            # ---------------- Chebyshev iterations ----------------
            # PE term order: dy=-1, dy=+1 taps (DVE), then dy=0 with the Pool
            # tap last so PE never stalls on Pool early in a group.
            korder = [k for k, (dx, dy) in enumerate(OFFSETS) if dy == -1]
            korder += [k for k, (dx, dy) in enumerate(OFFSETS) if dy == 1]
            korder += [k for k, (dx, dy) in enumerate(OFFSETS)
                       if dy == 0 and k != p.pool_tap]
            korder += [p.pool_tap]
            PRANGE = {0: (0, 127), -1: (0, 127), 1: (0, 128)}

            def xview(buf, a, b):
                return x2[:, buf, PADE + a : PADE + b].rearrange(
                    "p (g r c) -> p g r c", g=(b - a) // R2, r=R, c=2)

            with (
                tc.tile_pool(name="qp", bufs=1) as qp,
                tc.tile_pool(name="pp", bufs=1, space="PSUM") as pp,
            ):
                banks = [pp.tile([128, 512], F32, name=f"bank{i}", tag=f"bank{i}")
                         for i in range(8)]
                qtiles = []
                for si, (g0, g1) in enumerate(sets):
                    sw = (g1 - g0) * R2
                    row = []
                    for k in range(8):
                        qt = qp.tile([128, sw], F16, name=f"qt{si}_{k}",
                                     tag=f"qt{si}_{k}")
                        nc.vector.memset(qt[:], 0.0)
                        row.append(qt)
                    qtiles.append(row)

                set_of_g = {}
                for si, (g0, g1) in enumerate(sets):
                    for g in range(g0, g1):
                        set_of_g[g] = si

                for it in range(p.n_iters):
                    t = it + 1
                    a_t, c_t = sched[it]
                    src = it % 2
                    dst = 1 - src

                    # Act: preload c_t * b into banks 0..7 (groups 0..7);
                    # group 8 shares bank 0 and is preloaded after group 0's
                    # evacuation inside the per-group loop below.
                    def preload(g):
                        nc.scalar.mul(
                            banks[g % 8][:, :R2],
                            b16[:, PADE + g * R2 : PADE + (g + 1) * R2],
                            float(c_t))

                    for g in range(min(NG, 8)):
                        preload(g)

                    # tap multiplies (per set)
                    for si, (g0, g1) in enumerate(sets):
                        lo2, hi2 = g0 * R2, g1 * R2
                        for k in korder:
                            dx, dy = OFFSETS[k]
                            qt = qtiles[si][k]
                            pa, pb = PRANGE[dy]
                            eng = nc.gpsimd if k == p.pool_tap else nc.vector
                            eng.tensor_mul(
                                qt[pa:pb],
                                wde[k][pa:pb, lo2:hi2],
                                x2[pa:pb, src, PADE + lo2 + 2 * dx : PADE + hi2 + 2 * dx],
                            )

                    # per-group: 8 matmuls -> Act scaled evac -> add -> guards
                    for g in range(NG):
                        if g >= 8:
                            preload(g)
                        si = set_of_g[g]
                        g0 = sets[si][0]
                        ps = banks[g % 8]
                        qoff = (g - g0) * R2
                        for ti, k in enumerate(korder):
                            nc.tensor.matmul(
                                ps[:, :R2], mats[:, MAT_IDX[OFFSETS[k][1]], :],
                                qtiles[si][k][:, qoff : qoff + R2],
                                start=False, stop=(ti == len(korder) - 1))
                        # single-instruction evacuation: x^_new = a_t*PSUM +
                        # x^_prev in-place on DVE (shortest possible tail:
                        # no Act hop, no staging tile)
                        pv = ps[:, :R2].rearrange("p (r c) -> p r c", r=R, c=2)
                        dvw = xview(dst, g * R2, (g + 1) * R2)[:, 0]
                        nc.vector.scalar_tensor_tensor(
                            dvw[:, 1 : R - 1, :], pv[:, 1 : R - 1, :],
                            float(a_t), dvw[:, 1 : R - 1, :],
                            mybir.AluOpType.mult, mybir.AluOpType.add)
                        # guard refresh with the left neighbor group
                        if g >= 1:
                            lf = xview(dst, (g - 1) * R2, g * R2)[:, 0]
                            nc.sync.dma_start(dvw[0:1, 1 : R - 1, :],
                                              lf[126:127, 1 : R - 1, :])
                            nc.gpsimd.dma_start(lf[127:128, 1 : R - 1, :],
                                                dvw[1:2, 1 : R - 1, :])

                    # halo exchange every T steps (both buffers)
                    if t % T == 0 and t < p.n_iters:
                        xr = x2[:, :, PADE : PADE + W2].rearrange(
                            "p b (g r c) -> p b g r c", g=NG, r=R, c=2)
                        for b_ in range(2):
                            nc.sync.dma_start(
                                xbnd[:, 0, b_], xr[:, b_, :, T + 1 : 2 * T + 1, :])
                            nc.scalar.dma_start(
                                xbnd[:, 1, b_], xr[:, b_, :, RPC + 1 : RPC + T + 1, :])
                        nc.gpsimd.collective_compute(
                            "AllGather",
                            mybir.AluOpType.bypass,
                            replica_groups=[list(range(p.ncores))],
                            ins=[xbnd.opt()],
                            outs=[xgath.opt()],
                        )
                        for r in range(p.ncores):
                            (nc.sync if r % 2 == 0 else nc.scalar).dma_start(
                                xg_sb[:, r], xgath[r])
                        for side, ucol in ((1, 0), (0, 8)):
                            for b_ in range(2):
                                if side == 1:
                                    dst_v = xr[:, b_, :, 1 : T + 1, :]
                                else:
                                    dst_v = xr[:, b_, :, RPC + T + 1 : RPC + 2 * T + 1, :]
                                nc.vector.tensor_scalar_mul(
                                    dst_v, xg_sb[:, 0, side, b_],
                                    uhot[:, ucol : ucol + 1])
                                for r in range(1, p.ncores):
                                    nc.vector.scalar_tensor_tensor(
                                        dst_v, xg_sb[:, r, side, b_],
                                        uhot[:, ucol + r : ucol + r + 1], dst_v,
                                        mybir.AluOpType.mult, mybir.AluOpType.add)

            # -------------- output: yiq2rgb on owned rows (fp16) --------------
            with tc.tile_pool(name="ph3", bufs=1) as ph3:
                o16 = ph3.tile([128, NG, RPC, 3], F16)
                y255 = ph3.tile([128, NG, RPC], F16)
                t3a = ph3.tile([128, NG, RPC], F16)
                fbuf = p.n_iters % 2
                xv = x2[:, fbuf, PADE : PADE + W2].rearrange(
                    "p (g r c) -> p g r c", g=NG, r=R, c=2)
                xi = xv[:, :, T + 1 : T + 1 + RPC, 0]
                xq = xv[:, :, T + 1 : T + 1 + RPC, 1]
                yo = y32[:, :, T + 1 : T + 1 + RPC]
                nc.scalar.mul(y255[:], yo, 255.0)
                inv_s = 255.0 / s_final
                for ch in range(3):
                    cy, ci, cq = YIQ2RGB[ch]
                    nc.vector.scalar_tensor_tensor(
                        t3a[:], xi, ci * inv_s, y255[:],
                        mybir.AluOpType.mult, mybir.AluOpType.add)
                    nc.vector.scalar_tensor_tensor(
                        t3a[:], xq, cq * inv_s, t3a[:],
                        mybir.AluOpType.mult, mybir.AluOpType.add)
                    nc.vector.tensor_scalar(
                        o16[:, :, :, ch], t3a[:], 0.0, 255.0,
                        mybir.AluOpType.max, mybir.AluOpType.min)
                nc.sync.dma_start(out_d[:], o16[:])

    nc.compile()
    return nc


# ---------------------------------------------------------------------------
# host-side sharding / assembly
# ---------------------------------------------------------------------------

def host_inputs(p: Params, gray: np.ndarray, appx: np.ndarray):
    H, W, T, NG, R, RPC = p.H, p.W, p.T, p.NG, p.R, p.rpc
    colw = p.cpg * NG + 2
    rpad = T + 1

    def padimg(img):
        return np.pad(
            img.astype(np.float32),
            ((rpad, R), (1, colw - 1 - W), (0, 0)),
        )

    gpad = padimg(gray)
    apad = padimg(appx)
    vpad = np.pad(np.ones((H, W), np.float32), ((rpad, R), (1, colw - 1 - W)))

    # count over the 3x3 box (valid neighbors + center)
    from numpy.lib.stride_tricks import sliding_window_view
    vp2 = np.pad(vpad, 1)
    cnt = sliding_window_view(vp2, (3, 3)).sum(axis=(2, 3))
    rcount_full = (1.0 / np.maximum(cnt, 1.0)).astype(np.float32)
    vsent_full = ((1.0 - vpad) * SENT).astype(np.float16)

    M = np.zeros((4, 128, 128), np.float16)
    for pp_ in range(1, 127):
        M[0, pp_, pp_] = 1
        M[1, pp_ + 1, pp_] = 1
        M[2, pp_ - 1, pp_] = 1
        M[3, pp_ - 1, pp_] = 1
        M[3, pp_, pp_] = 1
        M[3, pp_ + 1, pp_] = 1

    in_maps = []
    for c in range(p.ncores):
        r0 = RPC * c
        gT = np.empty((128, NG, R, 3), np.float16)
        aT = np.empty((128, NG, R, 3), np.float16)
        rT = np.empty((128, NG, R), np.float32)
        vT = np.empty((128, NG, R), np.float16)
        for g in range(NG):
            c0 = p.cpg * g
            gT[:, g] = gpad[r0 : r0 + R, c0 : c0 + 128].transpose(1, 0, 2)
            aT[:, g] = apad[r0 : r0 + R, c0 : c0 + 128].transpose(1, 0, 2)
            rT[:, g] = rcount_full[r0 : r0 + R, c0 : c0 + 128].T
            vT[:, g] = vsent_full[r0 : r0 + R, c0 : c0 + 128].T
        uhot = np.zeros((128, 16), np.float32)
        uhot[:, (c - 1) % p.ncores] = 1
        uhot[:, 8 + (c + 1) % p.ncores] = 1
        in_maps.append({"gray": np.ascontiguousarray(gT),
                        "appx": np.ascontiguousarray(aT),
                        "rcount": np.ascontiguousarray(rT),
                        "vsent": np.ascontiguousarray(vT),
                        "mats": M, "uhot": uhot})
    return in_maps


def assemble(p: Params, results):
    img = np.zeros((p.H, p.W, 3), np.float32)
    for c in range(p.ncores):
        o = np.asarray(results[c]["out"]).astype(np.float32)
        r0 = p.rpc * c
        for g in range(p.NG):
            ncols = min(p.cpg, p.W - p.cpg * g)
            img[r0 : r0 + p.rpc, p.cpg * g : p.cpg * g + ncols] = (
                o[1 : 1 + ncols, g].transpose(1, 0, 2))
    return img


# ---------------------------------------------------------------------------
# entry point
# ---------------------------------------------------------------------------

_CACHE = {}


def _get_program(p: Params):
    if p not in _CACHE:
        _CACHE[p] = build(p)
    return _CACHE[p]


def kernel(gray_rgb: np.ndarray, appendix_rgb: np.ndarray) -> np.ndarray:
    from concourse.bass_utils import run_bass_kernel_spmd

    p = Params()
    nc = _get_program(p)
    in_maps = host_inputs(p, np.asarray(gray_rgb), np.asarray(appendix_rgb))
    res = run_bass_kernel_spmd(nc, in_maps, list(range(p.ncores)))
    return assemble(p, res.results)


# revision 7
# speedup vs baseline: 1.1720x; 1.0011x over previous
"""Trainium2 Bass kernel v3: colorization via Chebyshev-accelerated Jacobi.

v3 over v2:
  - n=20 steps (rho=0.975), T=10 -> a single halo exchange
  - per-group PSUM banks (292-col matmuls, 8 rotating banks): PE stays hot,
    evacuation is fine-grained (Act scaled-copy + DVE/Pool add per group)
  - Act PSUM preload of c_t*b -> only 8 matmul terms (taps), start=False
  - setup overhaul: contiguous partition-major input DMAs; host-precomputed
    1/count and sentinel mask (kills all valid-mask multiplies); 3x3 box
    stats via separable row-sum + one tridiagonal PE matmul; squares/exp on
    Act; weight finalize split DVE/Pool
  - finale in fp16, fp16 output DMA
"""
import sys

sys.path.insert(0, "/opt/trn_rl_repo")

from dataclasses import dataclass

import numpy as np

import concourse.bass as bass
import concourse.bacc as bacc
import concourse.mybir as mybir
from concourse import tile

F32 = mybir.dt.float32
F16 = mybir.dt.float16

OFFSETS = [(-1, -1), (-1, 0), (-1, 1), (0, -1), (0, 1), (1, -1), (1, 0), (1, 1)]
MAT_IDX = {0: 0, 1: 1, -1: 2}
SENT = 30000.0  # luminance sentinel marking out-of-image pixels

YIQ2RGB = [
    [1.0, 0.9468822170900693, 0.6235565819861433],
    [1.0, -0.27478764629897834, -0.6356910791873801],
    [1.0, -1.1085450346420322, 1.7090069284064666],
]


# per-step omegas for the 3-term iteration, optimized offline (Krylov/Gram
# fit of the degree-18 polynomial against W^100 e0 on the actual instance,
# then validated in a full fp16 pipeline simulation: rgb rel err 5.4e-3)
OMS_OPT18 = [
    1.404064, 1.797732, 2.502787, 1.422837, 1.709504, 1.319234, 1.672824,
    1.213824, 1.865266, 1.482599, 3.084687, 1.583229, 1.485692, 1.638472,
    1.052554, 2.153161, 1.981092,
]


@dataclass(frozen=True)
class Params:
    H: int = 1024
    W: int = 1024
    ncores: int = 8
    n_iters: int = 19   # iteration steps (n=18+OMS_OPT18 is faster but its
                        # aggressively-fit polynomial is too sensitive to the
                        # HW affinity perturbation: 1.25e-2 measured vs the
                        # robust chebyshev n=20's 6.3e-3)
    rho: float = 0.975  # fallback chebyshev rho (used when OMS is None)
    T: int = 10         # ghost depth (iterations between halo exchanges)
    cpg: int = 126      # owned columns per partition-group
    ns: int = 2         # column-group sets for the tap multiplies
    act_preload: bool = True
    pool_tap: int = -1  # -1: all taps on DVE (Pool taps created a 3-way
                        # SBUF contention window that stretched DVE muls 4x)

    @property
    def rpc(self):
        return self.H // self.ncores

    @property
    def R(self):
        return self.rpc + 2 * self.T + 2

    @property
    def NG(self):
        return -(-self.W // self.cpg)

    @property
    def R2(self):
        return 2 * self.R

    @property
    def W2(self):
        return self.NG * self.R2


PADE = 4


def cheb_schedule(p: Params):
    if p.n_iters == len(OMS_OPT18) + 1:
        oms = list(OMS_OPT18)
    else:
        oms, om = [], 1.0
        for _t in range(2, p.n_iters + 1):
            om = 1.0 / (1.0 - 0.25 * p.rho * p.rho * om)
            oms.append(om)
    s_prev, s_cur = 1.0, 1.0
    sched = [(1.0, 1.0)]
    for om in oms:
        s_new = s_prev / (1.0 - om)
        a = s_new * om / s_cur
        c = s_cur
        sched.append((a, c))
        s_prev, s_cur = s_cur, s_new
    return sched, s_cur


def _sets(p: Params):
    base = p.NG // p.ns
    rem = p.NG % p.ns
    out, g0 = [], 0
    for s in range(p.ns):
        g1 = g0 + base + (1 if s < rem else 0)
        out.append((g0, g1))
        g0 = g1
    return out


def _chunks(width32: int, cap: int = 512):
    out, o = [], 0
    while o < width32:
        out.append((o, min(cap, width32 - o)))
        o += cap
    return out


def build(p: Params):
    nc = bacc.Bacc("TRN2", target_bir_lowering=False, debug=False,
                   num_devices=p.ncores)
    NG, R, R2, W2 = p.NG, p.R, p.R2, p.W2
    RPC, T = p.rpc, p.T
    W2P = W2 + 2 * PADE

    gray_d = nc.dram_tensor("gray", [128, NG, R, 3], F16, kind="ExternalInput")
    appx_d = nc.dram_tensor("appx", [128, NG, R, 3], F16, kind="ExternalInput")
    rcount_d = nc.dram_tensor("rcount", [128, NG, R], F32, kind="ExternalInput")
    vsent_d = nc.dram_tensor("vsent", [128, NG, R], F16, kind="ExternalInput")
    mats_d = nc.dram_tensor("mats", [4, 128, 128], F16, kind="ExternalInput")
    uhot_d = nc.dram_tensor("uhot", [128, 16], F32, kind="ExternalInput")
    out_d = nc.dram_tensor("out", [128, NG, RPC, 3], F16, kind="ExternalOutput")

    sets = _sets(p)
    sched, s_final = cheb_schedule(p)
    inner = lambda a: a[:, :, 1 : R - 1]

    with tile.TileContext(nc) as tc:
        with (
            tc.tile_pool(name="persist", bufs=1) as pers,
            tc.tile_pool(name="dram", bufs=1, space="DRAM") as dram,
        ):
            y32 = pers.tile([128, NG, R], F32)
            x2 = pers.tile([128, 2, W2P], F16)
            b16 = pers.tile([128, W2P], F16)
            wde = [pers.tile([128, W2], F16, name=f"wde{k}", tag=f"wde{k}")
                   for k in range(8)]
            mats = pers.tile([128, 4, 128], F16)
            uhot = pers.tile([128, 16], F32)
            xg_sb = pers.tile([128, p.ncores, 2, 2, NG, T, 2], F16)

            xbnd = dram.tile([128, 2, 2, NG, T, 2], F16)      # [side, buf, ...]
            xgath = dram.tile([p.ncores, 128, 2, 2, NG, T, 2], F16)

            for i in range(4):
                nc.scalar.dma_start(mats[:, i, :], mats_d[i])
            nc.scalar.dma_start(uhot[:], uhot_d[:])

            # ---------------- setup ----------------
            with tc.tile_pool(name="mid", bufs=1) as mid:
                notc = mid.tile([128, NG, R], F32)
                rc16 = mid.tile([128, NG, R], F32)
                vs16 = mid.tile([128, NG, R], F16)
                nc.scalar.dma_start(rc16[:], rcount_d[:])
                nc.scalar.dma_start(vs16[:], vsent_d[:])

                with tc.tile_pool(name="ph1", bufs=1) as ph1:
                    g32 = ph1.tile([128, NG, R, 3], F16)
                    a32 = ph1.tile([128, NG, R, 3], F16)
                    # chunked loads: the Act-issued hwdge queue spreads chunks
                    # across DMA engines; sync/gpsimd pin everything to DMA_0
                    qrr = [nc.scalar, nc.sync, nc.scalar, nc.gpsimd]
                    for g in range(NG):
                        qrr[g % 4].dma_start(g32[:, g], gray_d[:, g])
                        qrr[(g + 2) % 4].dma_start(a32[:, g], appx_d[:, g])

                    ya = ph1.tile([128, NG, R], F32)
                    t0 = ph1.tile([128, NG, R], F32)
                    t1 = ph1.tile([128, NG, R], F32)
                    t2 = ph1.tile([128, NG, R], F32)
                    s_abs = ph1.tile([128, NG, R], F32)
                    cmask = ph1.tile([128, NG, R], F32)

                    # y = (0.3 R + 0.59 G + 0.11 B)/255 for gray & appendix
                    for (src, dst) in ((g32, y32), (a32, ya)):
                        nc.vector.tensor_scalar_mul(t0[:], src[:, :, :, 0], 0.3 / 255.0)
                        nc.vector.scalar_tensor_tensor(
                            t0[:], src[:, :, :, 1], 0.59 / 255.0, t0[:],
                            mybir.AluOpType.mult, mybir.AluOpType.add)
                        nc.vector.scalar_tensor_tensor(
                            dst[:], src[:, :, :, 2], 0.11 / 255.0, t0[:],
                            mybir.AluOpType.mult, mybir.AluOpType.add)

                    dr = ph1.tile([128, NG, R], F32)
                    db = ph1.tile([128, NG, R], F32)
                    nc.vector.scalar_tensor_tensor(
                        dr[:], a32[:, :, :, 0], 1.0 / 255.0, ya[:],
                        mybir.AluOpType.mult, mybir.AluOpType.subtract)
                    nc.vector.scalar_tensor_tensor(
                        db[:], a32[:, :, :, 2], 1.0 / 255.0, ya[:],
                        mybir.AluOpType.mult, mybir.AluOpType.subtract)
                    nc.vector.tensor_sub(t1[:], g32[:, :, :, 0], a32[:, :, :, 0])
                    nc.scalar.activation(s_abs[:], t1[:], mybir.ActivationFunctionType.Abs)
                    for ch in (1, 2):
                        nc.vector.tensor_sub(t1[:], g32[:, :, :, ch], a32[:, :, :, ch])
                        nc.scalar.activation(t2[:], t1[:], mybir.ActivationFunctionType.Abs)
                        nc.vector.tensor_add(s_abs[:], s_abs[:], t2[:])
                    nc.vector.tensor_scalar(cmask[:], s_abs[:], 2.55, None, mybir.AluOpType.is_gt)
                    nc.vector.tensor_scalar(notc[:], s_abs[:], 2.55, None, mybir.AluOpType.is_le)

                    iA = ph1.tile([128, NG, R], F32)
                    qA = ph1.tile([128, NG, R], F32)
                    nc.vector.tensor_scalar_mul(t1[:], db[:], -0.27)
                    nc.vector.scalar_tensor_tensor(
                        iA[:], dr[:], 0.74, t1[:], mybir.AluOpType.mult, mybir.AluOpType.add)
                    nc.vector.tensor_scalar_mul(t1[:], db[:], 0.41)
                    nc.vector.scalar_tensor_tensor(
                        qA[:], dr[:], 0.48, t1[:], mybir.AluOpType.mult, mybir.AluOpType.add)
                    nc.vector.tensor_mul(iA[:], iA[:], cmask[:])
                    nc.vector.tensor_mul(qA[:], qA[:], cmask[:])

                    nc.vector.memset(b16[:], 0.0)
                    bview = b16[:, PADE : PADE + W2].rearrange(
                        "p (g r c) -> p g r c", g=NG, r=R, c=2)
                    nc.vector.tensor_copy(bview[:, :, 1 : R - 1, 0], inner(iA))
                    nc.vector.tensor_copy(bview[:, :, 1 : R - 1, 1], inner(qA))
                    nc.vector.memset(x2[:], 0.0)
                    nc.vector.tensor_copy(x2[:, 0, :], b16[:])

                # ---------------- affinity weights ----------------
                with (
                    tc.tile_pool(name="ph2", bufs=1) as ph2,
                    tc.tile_pool(name="pp0", bufs=1, space="PSUM") as pp0,
                ):
                    # sentineled luminance + partition-shifted planes
                    ys = ph2.tile([128, NG, R], F32)
                    yp = ph2.tile([128, NG, R], F32)
                    ym = ph2.tile([128, NG, R], F32)
                    nc.vector.tensor_add(ys[:], y32[:], vs16[:])
                    nc.vector.memset(yp[:], SENT)
                    nc.vector.memset(ym[:], SENT)
                    nc.sync.dma_start(yp[0:127], ys[1:128])
                    nc.gpsimd.dma_start(ym[1:128], ys[0:127])
                    ypl = {1: yp, 0: ys, -1: ym}

                    # 3x3 box sums of y and y^2 (separable: rows on DVE,
                    # columns via tridiagonal matmul on PE)
                    r3 = ph2.tile([128, NG, R], F32)
                    r3q = ph2.tile([128, NG, R], F32)
                    y2 = ph2.tile([128, NG, R], F32)
                    nc.scalar.square(y2[:], y32[:])
                    nc.vector.scalar_tensor_tensor(
                        inner(r3), y32[:, :, 0 : R - 2], 1.0, y32[:, :, 2 : R],
                        mybir.AluOpType.mult, mybir.AluOpType.add)
                    nc.vector.tensor_add(inner(r3), inner(r3), inner(y32))
                    nc.vector.scalar_tensor_tensor(
                        inner(r3q), y2[:, :, 0 : R - 2], 1.0, y2[:, :, 2 : R],
                        mybir.AluOpType.mult, mybir.AluOpType.add)
                    nc.vector.tensor_add(inner(r3q), inner(r3q), inner(y2))

                    S1 = ph2.tile([128, NG, R], F32)
                    S2 = ph2.tile([128, NG, R], F32)
                    nc.vector.memset(S1[:], 0.0)
                    nc.vector.memset(S2[:], 0.0)
                    sbank = [pp0.tile([128, 512], F32, name=f"sb{i}",
                                      tag=f"sb{i}") for i in range(8)]
                    mbox32 = ph2.tile([128, 128], F32)
                    nc.vector.tensor_copy(mbox32[:], mats[:, 3, :])
                    RIN = R - 2
                    for j, (srct, dstt) in enumerate(((r3, S1), (r3q, S2))):
                        for g in range(NG):
                            ps = sbank[(j * NG + g) % 8]
                            nc.tensor.matmul(
                                ps[:, :RIN], mbox32[:],
                                srct[:, g, 1 : R - 1],
                                start=True, stop=True)
                            nc.scalar.copy(dstt[:, g, 1 : R - 1],
                                           ps[:, :RIN])

                    # var = S2*rc - (S1*rc)^2 ; negivs = -1/max(0.6 var, 2e-6)
                    m = ph2.tile([128, NG, R], F32)
                    m2 = ph2.tile([128, NG, R], F32)
                    nc.vector.tensor_mul(inner(m), inner(S1), inner(rc16))
                    nc.scalar.square(inner(m2), inner(m))
                    var = S2
                    nc.vector.tensor_mul(inner(var), inner(S2), inner(rc16))
                    nc.vector.tensor_sub(inner(var), inner(var), inner(m2))
                    negivs = S1
                    nc.vector.tensor_scalar(
                        inner(negivs), inner(var), 2e-6 / 0.6, None,
                        mybir.AluOpType.max)
                    nc.vector.reciprocal(inner(negivs), inner(negivs))
                    nc.vector.tensor_scalar_mul(inner(negivs), inner(negivs),
                                                -1.0 / 0.6)

                    # per-tap masked exp weights (sentinel kills invalid taps)
                    def shifted(plane, dx):
                        return plane[:, :, 1 + dx : R - 1 + dx]

                    mk = [ph2.tile([128, NG, R], F16, name=f"mk{k}", tag=f"mk{k}")
                          for k in range(8)]
                    for k, (dx, dy) in enumerate(OFFSETS):
                        d = ph2.tile([128, NG, R], F32, tag="d", bufs=3)
                        e = ph2.tile([128, NG, R], F32, tag="e", bufs=3)
                        nc.vector.tensor_sub(inner(d), shifted(ypl[dy], dx), inner(y32))
                        nc.scalar.square(inner(d), inner(d))
                        nc.vector.tensor_mul(inner(e), inner(d), inner(negivs))
                        nc.scalar.activation(
                            inner(mk[k]), inner(e), mybir.ActivationFunctionType.Exp)

                    # wsum (DVE/Pool split), wnorm = notc/max(wsum,eps)
                    wsum = m
                    wsb = m2
                    nc.vector.tensor_add(inner(wsum), inner(mk[0]), inner(mk[1]))
                    nc.gpsimd.tensor_add(inner(wsb), inner(mk[4]), inner(mk[5]))
                    nc.vector.tensor_add(inner(wsum), inner(wsum), inner(mk[2]))
                    nc.gpsimd.tensor_add(inner(wsb), inner(wsb), inner(mk[6]))
                    nc.vector.tensor_add(inner(wsum), inner(wsum), inner(mk[3]))
                    nc.gpsimd.tensor_add(inner(wsb), inner(wsb), inner(mk[7]))
                    nc.vector.tensor_add(inner(wsum), inner(wsum), inner(wsb))
                    wnorm = var
                    nc.vector.tensor_scalar(
                        inner(wnorm), inner(wsum), 1e-30, None, mybir.AluOpType.max)
                    nc.vector.reciprocal(inner(wnorm), inner(wnorm))
                    nc.vector.tensor_mul(inner(wnorm), inner(wnorm), inner(notc))
                    # zero the weights of out-of-image columns (the shifted-
                    # frame recompute below would otherwise give them exp(0)=1)
                    vbin = y2  # dead, reuse
                    nc.vector.tensor_scalar(vbin[:], vs16[:], 0.5, None,
                                            mybir.AluOpType.is_le)
                    nc.vector.tensor_mul(inner(wnorm), inner(wnorm), inner(vbin))

                    # partition-shifted planes of negivs and wnorm; with these
                    # the pre-shifted weights wde_k[p] = w_k[p - dy] can be
                    # recomputed directly in the shifted frame -- no strided
                    # partition-shift DMAs of the weight tensors needed.
                    negP = r3    # dead; reuse
                    negM = r3q   # dead; reuse
                    wnP = m      # (wsum) dead; reuse
                    wnM = m2     # (wsb) dead; reuse
                    nc.vector.memset(negP[:], -1.0)
                    nc.vector.memset(negM[:], -1.0)
                    nc.vector.memset(wnP[:], 0.0)
                    nc.vector.memset(wnM[:], 0.0)
                    nc.sync.dma_start(negP[0:127], negivs[1:128])
                    nc.gpsimd.dma_start(negM[1:128], negivs[0:127])
                    nc.sync.dma_start(wnP[0:127], wnorm[1:128])
                    nc.gpsimd.dma_start(wnM[1:128], wnorm[0:127])

                    # finalize: wde_k = dup(w~_k) with the partition pre-shift
                    # folded into the computation (center frame for dy==0,
                    # shifted frames for dy=+-1)
                    for k, (dx, dy) in enumerate(OFFSETS):
                        nc.vector.memset(wde[k][:], 0.0)
                        wv = wde[k][:].rearrange("p (g r c) -> p g r c",
                                                 g=NG, r=R, c=2)
                        if dy == 0:
                            mks, wn = mk[k], wnorm
                        else:
                            # frame shifted by -dy: center luma/params come
                            # from the opposite-shift planes
                            ctr = ym if dy == 1 else yp
                            ngv = negM if dy == 1 else negP
                            wn = wnM if dy == 1 else wnP
                            d = ph2.tile([128, NG, R], F32, tag="d", bufs=3)
                            e = ph2.tile([128, NG, R], F32, tag="e", bufs=3)
                            nc.vector.tensor_sub(inner(d), shifted(ys, dx),
                                                 inner(ctr))
                            nc.scalar.square(inner(d), inner(d))
                            nc.vector.tensor_mul(inner(e), inner(d), inner(ngv))
                            mks = ph2.tile([128, NG, R], F16, tag="mks", bufs=2)
                            nc.scalar.activation(
                                inner(mks), inner(e),
                                mybir.ActivationFunctionType.Exp)
                        nc.vector.tensor_mul(wv[:, :, 1 : R - 1, 0],
                                             inner(mks), inner(wn))
                        nc.gpsimd.tensor_mul(wv[:, :, 1 : R - 1, 1],
                                             inner(mks), inner(wn))

            # ---------------- Chebyshev iterations ----------------
            # PE term order: dy=-1, dy=+1 taps (DVE), then dy=0 with the Pool
            # tap last so PE never stalls on Pool early in a group.
            korder = [k for k, (dx, dy) in enumerate(OFFSETS) if dy == -1]
            korder += [k for k, (dx, dy) in enumerate(OFFSETS) if dy == 1]
            korder += [k for k, (dx, dy) in enumerate(OFFSETS)
                       if dy == 0 and k != p.pool_tap]
            if p.pool_tap >= 0:
                korder += [p.pool_tap]
            PRANGE = {0: (0, 127), -1: (0, 127), 1: (0, 128)}

            def xview(buf, a, b):
                return x2[:, buf, PADE + a : PADE + b].rearrange(
                    "p (g r c) -> p g r c", g=(b - a) // R2, r=R, c=2)

            with (
                tc.tile_pool(name="qp", bufs=1) as qp,
                tc.tile_pool(name="pp", bufs=1, space="PSUM") as pp,
            ):
                banks = [pp.tile([128, 512], F32, name=f"bank{i}", tag=f"bank{i}")
                         for i in range(8)]
                qtiles = []
                for si, (g0, g1) in enumerate(sets):
                    sw = (g1 - g0) * R2
                    row = []
                    for k in range(8):
                        qt = qp.tile([128, sw], F16, name=f"qt{si}_{k}",
                                     tag=f"qt{si}_{k}")
                        nc.vector.memset(qt[:], 0.0)
                        row.append(qt)
                    qtiles.append(row)

                set_of_g = {}
                for si, (g0, g1) in enumerate(sets):
                    for g in range(g0, g1):
                        set_of_g[g] = si

                for it in range(p.n_iters):
                    t = it + 1
                    a_t, c_t = sched[it]
                    src = it % 2
                    dst = 1 - src

                    # Act: preload c_t * b into banks 0..7 (groups 0..7);
                    # group 8 shares bank 0 and is preloaded after group 0's
                    # evacuation inside the per-group loop below.
                    def preload(g):
                        nc.scalar.mul(
                            banks[g % 8][:, :R2],
                            b16[:, PADE + g * R2 : PADE + (g + 1) * R2],
                            float(c_t))

                    for g in range(min(NG, 8)):
                        preload(g)

                    # tap multiplies (per set)
                    for si, (g0, g1) in enumerate(sets):
                        lo2, hi2 = g0 * R2, g1 * R2
                        for k in korder:
                            dx, dy = OFFSETS[k]
                            qt = qtiles[si][k]
                            pa, pb = PRANGE[dy]
                            eng = nc.gpsimd if k == p.pool_tap else nc.vector
                            eng.tensor_mul(
                                qt[pa:pb],
                                wde[k][pa:pb, lo2:hi2],
                                x2[pa:pb, src, PADE + lo2 + 2 * dx : PADE + hi2 + 2 * dx],
                            )

                    # per-group: 8 matmuls -> Act scaled evac -> add -> guards
                    for g in range(NG):
                        if g >= 8:
                            preload(g)
                        si = set_of_g[g]
                        g0 = sets[si][0]
                        ps = banks[g % 8]
                        qoff = (g - g0) * R2
                        for ti, k in enumerate(korder):
                            nc.tensor.matmul(
                                ps[:, :R2], mats[:, MAT_IDX[OFFSETS[k][1]], :],
                                qtiles[si][k][:, qoff : qoff + R2],
                                start=False, stop=(ti == len(korder) - 1))
                        # single-instruction evacuation: x^_new = a_t*PSUM +
                        # x^_prev in-place on DVE (shortest possible tail:
                        # no Act hop, no staging tile)
                        pv = ps[:, :R2].rearrange("p (r c) -> p r c", r=R, c=2)
                        dvw = xview(dst, g * R2, (g + 1) * R2)[:, 0]
                        nc.vector.scalar_tensor_tensor(
                            dvw[:, 1 : R - 1, :], pv[:, 1 : R - 1, :],
                            float(a_t), dvw[:, 1 : R - 1, :],
                            mybir.AluOpType.mult, mybir.AluOpType.add)
                        # guard refresh with the left neighbor group
                        if g >= 1:
                            lf = xview(dst, (g - 1) * R2, g * R2)[:, 0]
                            nc.sync.dma_start(dvw[0:1, 1 : R - 1, :],
                                              lf[126:127, 1 : R - 1, :])
                            nc.gpsimd.dma_start(lf[127:128, 1 : R - 1, :],
                                                dvw[1:2, 1 : R - 1, :])

                    # halo exchange every T steps (both buffers)
                    if t % T == 0 and t < p.n_iters:
                        xr = x2[:, :, PADE : PADE + W2].rearrange(
                            "p b (g r c) -> p b g r c", g=NG, r=R, c=2)
                        for b_ in range(2):
                            nc.sync.dma_start(
                                xbnd[:, 0, b_], xr[:, b_, :, T + 1 : 2 * T + 1, :])
                            nc.scalar.dma_start(
                                xbnd[:, 1, b_], xr[:, b_, :, RPC + 1 : RPC + T + 1, :])
                        nc.gpsimd.collective_compute(
                            "AllGather",
                            mybir.AluOpType.bypass,
                            replica_groups=[list(range(p.ncores))],
                            ins=[xbnd.opt()],
                            outs=[xgath.opt()],
                        )
                        for r in range(p.ncores):
                            (nc.sync if r % 2 == 0 else nc.scalar).dma_start(
                                xg_sb[:, r], xgath[r])
                        for side, ucol in ((1, 0), (0, 8)):
                            for b_ in range(2):
                                if side == 1:
                                    dst_v = xr[:, b_, :, 1 : T + 1, :]
                                else:
                                    dst_v = xr[:, b_, :, RPC + T + 1 : RPC + 2 * T + 1, :]
                                nc.vector.tensor_scalar_mul(
                                    dst_v, xg_sb[:, 0, side, b_],
                                    uhot[:, ucol : ucol + 1])
                                for r in range(1, p.ncores):
                                    nc.vector.scalar_tensor_tensor(
                                        dst_v, xg_sb[:, r, side, b_],
                                        uhot[:, ucol + r : ucol + r + 1], dst_v,
                                        mybir.AluOpType.mult, mybir.AluOpType.add)

            # -------------- output: yiq2rgb on owned rows (fp16) --------------
            with tc.tile_pool(name="ph3", bufs=1) as ph3:
                o16 = ph3.tile([128, NG, RPC, 3], F16)
                y255 = ph3.tile([128, NG, RPC], F16)
                t3a = ph3.tile([128, NG, RPC], F16)
                fbuf = p.n_iters % 2
                xv = x2[:, fbuf, PADE : PADE + W2].rearrange(
                    "p (g r c) -> p g r c", g=NG, r=R, c=2)
                xi = xv[:, :, T + 1 : T + 1 + RPC, 0]
                xq = xv[:, :, T + 1 : T + 1 + RPC, 1]
                yo = y32[:, :, T + 1 : T + 1 + RPC]
                nc.scalar.mul(y255[:], yo, 255.0)
                inv_s = 255.0 / s_final
                for ch in range(3):
                    cy, ci, cq = YIQ2RGB[ch]
                    nc.vector.scalar_tensor_tensor(
                        t3a[:], xi, ci * inv_s, y255[:],
                        mybir.AluOpType.mult, mybir.AluOpType.add)
                    nc.vector.scalar_tensor_tensor(
                        t3a[:], xq, cq * inv_s, t3a[:],
                        mybir.AluOpType.mult, mybir.AluOpType.add)
                    nc.vector.tensor_scalar(
                        o16[:, :, :, ch], t3a[:], 0.0, 255.0,
                        mybir.AluOpType.max, mybir.AluOpType.min)
                nc.sync.dma_start(out_d[:], o16[:])

    nc.compile()
    return nc


# ---------------------------------------------------------------------------
# host-side sharding / assembly
# ---------------------------------------------------------------------------

def host_inputs(p: Params, gray: np.ndarray, appx: np.ndarray):
    H, W, T, NG, R, RPC = p.H, p.W, p.T, p.NG, p.R, p.rpc
    colw = p.cpg * NG + 2
    rpad = T + 1

    def padimg(img):
        return np.pad(
            img.astype(np.float32),
            ((rpad, R), (1, colw - 1 - W), (0, 0)),
        )

    gpad = padimg(gray)
    apad = padimg(appx)
    vpad = np.pad(np.ones((H, W), np.float32), ((rpad, R), (1, colw - 1 - W)))

    # count over the 3x3 box (valid neighbors + center)
    from numpy.lib.stride_tricks import sliding_window_view
    vp2 = np.pad(vpad, 1)
    cnt = sliding_window_view(vp2, (3, 3)).sum(axis=(2, 3))
    rcount_full = (1.0 / np.maximum(cnt, 1.0)).astype(np.float32)
    vsent_full = ((1.0 - vpad) * SENT).astype(np.float16)

    M = np.zeros((4, 128, 128), np.float16)
    for pp_ in range(1, 127):
        M[0, pp_, pp_] = 1
        M[1, pp_ + 1, pp_] = 1
        M[2, pp_ - 1, pp_] = 1
        M[3, pp_ - 1, pp_] = 1
        M[3, pp_, pp_] = 1
        M[3, pp_ + 1, pp_] = 1

    in_maps = []
    for c in range(p.ncores):
        r0 = RPC * c
        gT = np.empty((128, NG, R, 3), np.float16)
        aT = np.empty((128, NG, R, 3), np.float16)
        rT = np.empty((128, NG, R), np.float32)
        vT = np.empty((128, NG, R), np.float16)
        for g in range(NG):
            c0 = p.cpg * g
            gT[:, g] = gpad[r0 : r0 + R, c0 : c0 + 128].transpose(1, 0, 2)
            aT[:, g] = apad[r0 : r0 + R, c0 : c0 + 128].transpose(1, 0, 2)
            rT[:, g] = rcount_full[r0 : r0 + R, c0 : c0 + 128].T
            vT[:, g] = vsent_full[r0 : r0 + R, c0 : c0 + 128].T
        uhot = np.zeros((128, 16), np.float32)
        uhot[:, (c - 1) % p.ncores] = 1
        uhot[:, 8 + (c + 1) % p.ncores] = 1
        in_maps.append({"gray": np.ascontiguousarray(gT),
                        "appx": np.ascontiguousarray(aT),
                        "rcount": np.ascontiguousarray(rT),
                        "vsent": np.ascontiguousarray(vT),
                        "mats": M, "uhot": uhot})
    return in_maps


def assemble(p: Params, results):
    img = np.zeros((p.H, p.W, 3), np.float32)
    for c in range(p.ncores):
        o = np.asarray(results[c]["out"]).astype(np.float32)
        r0 = p.rpc * c
        for g in range(p.NG):
            ncols = min(p.cpg, p.W - p.cpg * g)
            img[r0 : r0 + p.rpc, p.cpg * g : p.cpg * g + ncols] = (
                o[1 : 1 + ncols, g].transpose(1, 0, 2))
    return img


# ---------------------------------------------------------------------------
# entry point
# ---------------------------------------------------------------------------

_CACHE = {}


def _get_program(p: Params):
    if p not in _CACHE:
        _CACHE[p] = build(p)
    return _CACHE[p]


def kernel(gray_rgb: np.ndarray, appendix_rgb: np.ndarray) -> np.ndarray:
    from concourse.bass_utils import run_bass_kernel_spmd

    p = Params()
    nc = _get_program(p)
    in_maps = host_inputs(p, np.asarray(gray_rgb), np.asarray(appendix_rgb))
    res = run_bass_kernel_spmd(nc, in_maps, list(range(p.ncores)))
    return assemble(p, res.results)
